# revision 18
# baseline (speedup 1.0000x reference)
"""KNN grouped-vector-attention pool kernel for 8 Trainium2 NeuronCores.

Strategy: shard queries M=16384 across 8 cores (2048 each). The context
feature table is sharded across cores and reassembled on device with an HBM
AllGather; each core then resolves its own KNN gathers locally via indirect
DMA and XBAR DMA-transposes into channel-major layout.

Interconnect to the device is the bottleneck (slow tunneled PJRT link), so
the wire format is aggressively compressed:
  * context rows ship as 132-byte records: 128 x int8 (per-row absmax
    quantized) + the fp16 row scale + 2B pad. Dequant happens on device
    right after each 128-row gather (ACT engine, per-partition scale).
    The AllGather moves the records typed as u16: the collective datapath
    routes elements through f32, so u8/u16 round-trip exactly while f16
    wires flush denormals and i32 wires lose low mantissa bits.
  * relative positions ship as int8 with a per-core global scale folded
    into the positional-BN scale scalar.
  * the output returns as uint8 (per-channel absmax quantized, bias +128)
    with the per-channel fp32 scale appended to the row.
  * the query projection is rank-reduced on the host: q only enters the
    logits as q@Ww1 [M,8], so an [8,M] fp16 stripe uploads instead of the
    [128,M] query block.
  * Ww2 uploads as a single 8x8 tile; its 16-block block-diagonal expansion
    is assembled on device. Sel / E / Ww1s / WpW1s selector matrices are
    synthesized on device from tiny seeds.
All per-core inputs pack into one contiguous fp16 blob (~2.4MB) so the
host->device path pays a single transfer per core. The dispatch path is a
custom PJRT runner that keeps the donated-zero output buffers device-
resident, avoiding run_bass_kernel_spmd's per-call zero upload. All matmuls
run fp16 x fp16 with fp32 PSUM accumulation.
"""
import sys
sys.path.insert(0, '/opt/trn_rl_repo')
import numpy as np

N_CORES = 8
M, N, K, C, G = 16384, 131072, 16, 128, 8
M_LOC = M // N_CORES          # 2048 queries per core
R_LOC = M_LOC * K             # 32768 gathered rows per core
N_LOC = N // N_CORES          # 16384 context rows uploaded per core
CHUNK = 512                   # rows per compute chunk (one PSUM bank)
GROUP = 16 * CHUNK            # 8192 rows per stacked group
N_GROUPS = R_LOC // GROUP     # 4
ROWB = 132                    # context record: 128 int8 + fp16 scale + pad
EPS_BN = 1e-5

# blob layout: (name, (partitions, cols)) packed row-major, fp16 units
_LAYOUT = [
    ("ctxtab", (C, N_LOC * ROWB // 2 // C)),  # [N_LOC,132] u8 records
    ("nqwT", (G, M_LOC)),        # -(relu(bn(qf@Wq)) @ Ww1).T : rank-8 q path
    ("Wk", (C, C)), ("Wv", (C, C)), ("Wp2", (C, C)),
    ("Ww2s", (G, G)),
    ("Ww1", (C, G)), ("P2W1", (C, G)),
    ("scal", (C, 9)),            # sq,bq,sk,bk,bv,sp1,bp1,sw1,bw1
    ("knn16", (C, R_LOC // C * 2)),  # [128,256] i32 KNN blocks, fp16 bits
    ("posT8", (3, R_LOC // 2)),  # [3, R_LOC] int8, fp16 bits
    ("Wp1", (3, C)),
]
_OFFS = {}
_NTOT = 0
for _nm, (_p, _c) in _LAYOUT:
    _OFFS[_nm] = _NTOT
    _NTOT += _p * _c

_compiled = None


def _build():
    from concourse import bacc, bass, mybir
    import concourse.tile as tile

    f32 = mybir.dt.float32
    f16 = mybir.dt.float16
    i32 = mybir.dt.int32
    i8 = mybir.dt.int8
    u8 = mybir.dt.uint8
    u16 = mybir.dt.uint16
    AF = mybir.ActivationFunctionType
    OP = mybir.AluOpType

    nc = bacc.Bacc("TRN2", target_bir_lowering=False, debug=False,
                   num_devices=N_CORES)

    blob = nc.dram_tensor("blob", (_NTOT,), f16, kind="ExternalInput").ap()
    out_d = nc.dram_tensor("out", (C, M_LOC + 4), u8,
                           kind="ExternalOutput").ap()

    def view(nm):
        p, c = dict(_LAYOUT)[nm]
        off = _OFFS[nm]
        return blob[off:off + p * c].rearrange("(p c) -> p c", p=p)

    from contextlib import ExitStack
    est = ExitStack()
    with tile.TileContext(nc) as tc, est:
        dpool = est.enter_context(tc.tile_pool(name="dram", bufs=1, space="DRAM"))
        cpool = est.enter_context(tc.tile_pool(name="const", bufs=1))
        gtpool = est.enter_context(tc.tile_pool(name="gt", bufs=8))
        g16pool = est.enter_context(tc.tile_pool(name="g16", bufs=8))
        scpool = est.enter_context(tc.tile_pool(name="gsc", bufs=8))
        gpool = est.enter_context(tc.tile_pool(name="gath", bufs=2))
        p8pool = est.enter_context(tc.tile_pool(name="p8st", bufs=2))
        vpool = est.enter_context(tc.tile_pool(name="valp", bufs=2))
        spool = est.enter_context(tc.tile_pool(name="work", bufs=2))
        opool = est.enter_context(tc.tile_pool(name="outp", bufs=1))
        ps = {}
        for nm, nb in [("kp", 2), ("px", 2), ("vp", 2), ("stk", 1), ("wr", 1)]:
            ps[nm] = est.enter_context(tc.tile_pool(name=nm, bufs=nb, space="PSUM"))

        # ---- AllGather the context record table in HBM ----------------
        # wire dtype MUST be an int type <= 16 bits: the collective datapath
        # routes elements through f32 (u16/u8 round-trip exactly; f16
        # denormals get flushed and i32 loses low mantissa bits)
        tp, tc_ = dict(_LAYOUT)["ctxtab"]
        ib = dpool.tile([tp, tc_], u16, tag="ib", name="ib")
        ob = dpool.tile([tp, tc_ * N_CORES], u16, tag="ob", name="ob",
                        addr_space="Shared")
        nc.gpsimd.dma_start(ib[:], view("ctxtab").bitcast(u16))
        nc.gpsimd.collective_compute(
            "AllGather", OP.bypass,
            replica_groups=[list(range(N_CORES))],
            ins=[ib.opt()], outs=[ob.opt()])
        # reinterpret the gathered flat buffer as [N, 132] u8 records
        ctx2d = ob[:].bitcast(u8).rearrange("p (r c) -> (p r) c", c=ROWB)

        # ---- constants into SBUF -------------------------------------
        ct = {}
        for nm in ("nqwT", "Wk", "Wv", "Wp2", "Ww2s", "Ww1", "P2W1",
                   "scal", "knn16", "Wp1"):
            p, c = dict(_LAYOUT)[nm]
            ct[nm] = cpool.tile([p, c], f16, tag=f"c_{nm}", name=f"c_{nm}")
            nc.sync.dma_start(out=ct[nm][:], in_=view(nm))
        knn32 = ct["knn16"][:].bitcast(i32)          # [128, R_LOC/128] i32
        # fp16 scalars -> f32 working copy; per-scalar column APs
        scal32 = cpool.tile([C, 9], f32, tag="c_scal32", name="c_scal32")
        nc.vector.tensor_copy(out=scal32[:], in_=ct["scal"][:])
        for j, nm in enumerate(("sq", "bq", "sk", "bk", "bv", "sp1", "bp1",
                                "sw1", "bw1")):
            ct[nm] = scal32[:, j:j + 1]

        # W2bd: 16-block block-diagonal expansion of Ww2 (8x8)
        w2bd = cpool.tile([C, C], f16, tag="c_w2bd", name="c_w2bd")
        nc.gpsimd.memset(w2bd[:], 0.0)
        for i in range(16):
            nc.sync.dma_start(out=w2bd[8 * i:8 * i + 8, 8 * i:8 * i + 8],
                              in_=ct["Ww2s"][:])

        # ---- synthesize Sel / Ww1s / WpW1s on device -----------------
        # Sel[p, j] = 1 iff j // 16 == p  (i.e. 0 <= j - 16p <= 15)
        sel = cpool.tile([C, 16 * C], f16, tag="c_sel", name="c_sel")
        nc.gpsimd.memset(sel[:], 1.0)
        nc.gpsimd.affine_select(out=sel[:], in_=sel[:], compare_op=OP.is_ge,
                                fill=0.0, base=0, pattern=[[1, 16 * C]],
                                channel_multiplier=-16)
        nc.gpsimd.affine_select(out=sel[:], in_=sel[:], compare_op=OP.is_gt,
                                fill=0.0, base=16, pattern=[[-1, 16 * C]],
                                channel_multiplier=16)
        # E block i maps the 8 q-logit rows onto stacked partitions 8i+g
        etile = cpool.tile([G, 16 * C], f16, tag="c_e", name="c_e")
        nc.gpsimd.memset(etile[:], 0.0)
        nc.gpsimd.affine_select(out=etile[:], in_=etile[:],
                                compare_op=OP.not_equal, fill=1.0, base=0,
                                pattern=[[-8, 16], [1, C]],
                                channel_multiplier=-1)
        # Ww1s block i holds Ww1 at cols i*C + 8i .. +8 (rest zero)
        ww1s = cpool.tile([C, 16 * C], f16, tag="c_ww1s", name="c_ww1s")
        wpw1s = cpool.tile([C, 16 * C], f16, tag="c_wpw1s", name="c_wpw1s")
        nc.gpsimd.memset(ww1s[:], 0.0)
        nc.gpsimd.memset(wpw1s[:], 0.0)
        for i in range(16):
            c0 = i * C + 8 * i
            nc.vector.tensor_copy(out=ww1s[:, c0:c0 + 8], in_=ct["Ww1"][:])
            nc.vector.tensor_copy(out=wpw1s[:, c0:c0 + 8], in_=ct["P2W1"][:])

        outT = opool.tile([C, M_LOC], f32)

        for g in range(N_GROUPS):
            fT = gpool.tile([C, GROUP], f16, tag="fT")
            # gather + dequant + transpose this group's 8192 neighbor rows
            for blk in range(GROUP // C):
                gcol = g * (GROUP // C) + blk
                gt = gtpool.tile([C, ROWB], u8, tag="gt")
                nc.gpsimd.indirect_dma_start(
                    out=gt[:], out_offset=None,
                    in_=ctx2d,
                    in_offset=bass.IndirectOffsetOnAxis(
                        ap=knn32[:, gcol:gcol + 1], axis=0))
                sc32 = scpool.tile([C, 1], f32, tag="gsc")
                nc.vector.tensor_copy(out=sc32[:],
                                      in_=gt[:, 128:130].bitcast(f16))
                gt16 = g16pool.tile([C, C], f16, tag="g16")
                nc.scalar.activation(out=gt16[:], in_=gt[:, 0:128].bitcast(i8),
                                     func=AF.Identity, bias=0.0,
                                     scale=sc32[:])
                eng = nc.sync if blk % 2 == 0 else nc.scalar
                eng.dma_start_transpose(
                    out=fT[:, blk * C:(blk + 1) * C], in_=gt16[:])
            pT8 = p8pool.tile([3, GROUP // 2], f16, tag="pT8")
            nc.sync.dma_start(
                out=pT8[:],
                in_=view("posT8")[:, g * (GROUP // 2):(g + 1) * (GROUP // 2)])
            pT = gpool.tile([3, GROUP], f16, tag="pT")
            nc.vector.tensor_copy(out=pT[:], in_=pT8[:].bitcast(i8))
            valT = vpool.tile([C, GROUP], f32, tag="valp")
            stacked_ps = ps["stk"].tile([C, CHUNK], f32, tag="stk_t", name="stacked_ps")
            # -------- phase A: per chunk of 512 gathered rows ---------
            for i in range(16):
                ch = g * 16 + i              # global chunk id
                q0 = ch * 32                 # first query of chunk
                ctx = fT[:, i * CHUNK:(i + 1) * CHUNK]
                pos = pT[:, i * CHUNK:(i + 1) * CHUNK]
                # key = relu(bn(Wk.T @ ctx))
                k_ps = ps["kp"].tile([C, CHUNK], f32, tag="kp_t", name="k_ps")
                nc.tensor.matmul(out=k_ps[:], lhsT=ct["Wk"][:], rhs=ctx,
                                 start=True, stop=True)
                keyT = spool.tile([C, CHUNK], f16, tag="keyT")
                nc.scalar.activation(out=keyT[:], in_=k_ps[:], func=AF.Relu,
                                     bias=ct["bk"], scale=ct["sk"])
                # pebx = relu(bn(Wp1.T @ pos))
                pebx_ps = ps["px"].tile([C, CHUNK], f32, tag="px_t", name="pebx_ps")
                nc.tensor.matmul(out=pebx_ps[:], lhsT=ct["Wp1"][:], rhs=pos,
                                 start=True, stop=True)
                pebxT = spool.tile([C, CHUNK], f16, tag="pebxT")
                nc.scalar.activation(out=pebxT[:], in_=pebx_ps[:], func=AF.Relu,
                                     bias=ct["bp1"], scale=ct["sp1"])
                # val = Wv.T @ ctx + Wp2.T @ pebx (+ bv + bp2 via bias)
                v_ps = ps["vp"].tile([C, CHUNK], f32, tag="vp_t", name="v_ps")
                nc.tensor.matmul(out=v_ps[:], lhsT=ct["Wv"][:], rhs=ctx,
                                 start=True, stop=False)
                nc.tensor.matmul(out=v_ps[:], lhsT=ct["Wp2"][:], rhs=pebxT[:],
                                 start=False, stop=True)
                nc.scalar.activation(out=valT[:, i * CHUNK:(i + 1) * CHUNK],
                                     in_=v_ps[:], func=AF.Identity,
                                     bias=ct["bv"], scale=1.0)
                # w1 logits, stacked: Ww1.T @ (key - q + peb) with
                # peb folded via WpW1s = Wp2 @ Ww1s and -q via nqT
                q_rep = ct["nqwT"][:, q0:q0 + 32].unsqueeze(2) \
                    .to_broadcast([G, 32, K])
                nc.tensor.matmul(out=stacked_ps[:],
                                 lhsT=ww1s[:, i * C:(i + 1) * C],
                                 rhs=keyT[:], start=(i == 0), stop=False,
                                 skip_group_check=True)
                nc.tensor.matmul(out=stacked_ps[:],
                                 lhsT=wpw1s[:, i * C:(i + 1) * C],
                                 rhs=pebxT[:], start=False, stop=False,
                                 skip_group_check=True)
                nc.tensor.matmul(out=stacked_ps[:],
                                 lhsT=etile[:, i * C:(i + 1) * C],
                                 rhs=q_rep, start=False, stop=(i == 15),
                                 skip_group_check=True)
            # -------- group tail: bn/relu, mm2, softmax ---------------
            stk_bn = spool.tile([C, CHUNK], f16, tag="stkbn")
            nc.scalar.activation(out=stk_bn[:], in_=stacked_ps[:], func=AF.Relu,
                                 bias=ct["bw1"], scale=ct["sw1"])
            w2_ps = ps["px"].tile([C, CHUNK], f32, tag="px_t", name="w2_ps")
            nc.tensor.matmul(out=w2_ps[:], lhsT=w2bd[:], rhs=stk_bn[:],
                             start=True, stop=True)
            mx = spool.tile([C, 32], f32, tag="mx")
            nc.vector.tensor_reduce(
                out=mx[:], in_=w2_ps[:].rearrange("p (m k) -> p m k", k=K),
                axis=mybir.AxisListType.X, op=OP.max)
            sm = spool.tile([C, CHUNK], f32, tag="sm")
            nc.vector.tensor_tensor(
                out=sm[:].rearrange("p (m k) -> p m k", k=K),
                in0=w2_ps[:].rearrange("p (m k) -> p m k", k=K),
                in1=mx[:].unsqueeze(2).to_broadcast([C, 32, K]),
                op=OP.subtract)
            e_t = spool.tile([C, CHUNK], f32, tag="e")
            nc.scalar.activation(out=e_t[:], in_=sm[:], func=AF.Exp)
            s_t = spool.tile([C, 32], f32, tag="s")
            nc.vector.tensor_reduce(
                out=s_t[:], in_=e_t[:].rearrange("p (m k) -> p m k", k=K),
                axis=mybir.AxisListType.X, op=OP.add)
            rinv = spool.tile([C, 32], f32, tag="rinv")
            nc.vector.reciprocal(out=rinv[:], in_=s_t[:])
            wf32 = spool.tile([C, CHUNK], f32, tag="wf32")
            nc.vector.tensor_tensor(
                out=wf32[:].rearrange("p (m k) -> p m k", k=K),
                in0=e_t[:].rearrange("p (m k) -> p m k", k=K),
                in1=rinv[:].unsqueeze(2).to_broadcast([C, 32, K]),
                op=OP.mult)
            wfin = spool.tile([C, CHUNK], f16, tag="wfin")
            nc.scalar.activation(out=wfin[:], in_=wf32[:], func=AF.Identity)
            # -------- phase B: weighted sum per chunk -----------------
            for i in range(16):
                ch = g * 16 + i
                wrep_ps = ps["wr"].tile([C, CHUNK], f32, tag="wr_t", name="wrep_ps")
                nc.tensor.matmul(out=wrep_ps[:],
                                 lhsT=sel[:, i * C:(i + 1) * C],
                                 rhs=wfin[:], start=True, stop=True)
                prod = spool.tile([C, CHUNK], f32, tag="prod")
                nc.vector.tensor_tensor(out=prod[:],
                                        in0=valT[:, i * CHUNK:(i + 1) * CHUNK],
                                        in1=wrep_ps[:], op=OP.mult)
                nc.vector.tensor_reduce(
                    out=outT[:, ch * 32:(ch + 1) * 32],
                    in_=prod[:].rearrange("p (m k) -> p m k", k=K),
                    axis=mybir.AxisListType.X, op=OP.add)

        # -------- output: per-channel u8 quantization -----------------
        m_t = opool.tile([C, 1], f32, tag="omax", name="omax")
        mn_t = opool.tile([C, 1], f32, tag="omin", name="omin")
        nc.vector.tensor_reduce(out=m_t[:], in_=outT[:],
                                axis=mybir.AxisListType.X, op=OP.max)
        nc.vector.tensor_reduce(out=mn_t[:], in_=outT[:],
                                axis=mybir.AxisListType.X, op=OP.min)
        nc.vector.tensor_scalar_mul(out=mn_t[:], in0=mn_t[:], scalar1=-1.0)
        nc.vector.tensor_tensor(out=m_t[:], in0=m_t[:], in1=mn_t[:],
                                op=OP.max)
        nc.vector.tensor_scalar_max(out=m_t[:], in0=m_t[:], scalar1=1e-20)
        rs = opool.tile([C, 1], f32, tag="orsc", name="orsc")
        nc.vector.reciprocal(out=rs[:], in_=m_t[:])
        # 126.99 (not 127) so the row-max element lands strictly below
        # 255.5 after the +128.5 shift even if reciprocal rounds up --
        # keeps the u8 convert away from any wrap/saturate edge.
        nc.vector.tensor_scalar_mul(out=rs[:], in0=rs[:], scalar1=126.99)
        y8 = opool.tile([C, M_LOC], u8, tag="oy8", name="oy8")
        nc.vector.tensor_scalar(out=y8[:], in0=outT[:], scalar1=rs[:],
                                scalar2=128.5, op0=OP.mult, op1=OP.add)
        nc.sync.dma_start(out=out_d[:, 0:M_LOC], in_=y8[:])
        nc.sync.dma_start(out=out_d[:, M_LOC:M_LOC + 4].bitcast(f32),
                          in_=m_t[:])

    nc.compile()
    return nc


def _make_runner(nc):
    """PJRT dispatch for the compiled Bass module, mirroring
    bass2jax.run_bass_via_pjrt but keeping the (ignored) donated-zero
    output operands device-resident so each call uploads only the blob."""
    import jax
    from jax.sharding import Mesh, PartitionSpec, NamedSharding
    from jax.experimental.shard_map import shard_map
    from concourse import bass2jax, mybir

    bass2jax.install_neuronx_cc_hook()
    assert nc.dbg_addr is None or not nc.dbg_callbacks

    partition_name = (nc.partition_id_tensor.name
                      if nc.partition_id_tensor else None)
    in_names, in_shapes = [], {}
    out_names, out_avals, zero_outs = [], [], []
    for alloc in nc.m.functions[0].allocations:
        if not isinstance(alloc, mybir.MemoryLocationSet):
            continue
        name = alloc.memorylocations[0].name
        if alloc.kind == "ExternalInput":
            if name != partition_name:
                in_names.append(name)
                in_shapes[name] = (tuple(alloc.tensor_shape),
                                   mybir.dt.np(alloc.dtype))
        elif alloc.kind == "ExternalOutput":
            shape = tuple(alloc.tensor_shape)
            dtype = mybir.dt.np(alloc.dtype)
            out_names.append(name)
            out_avals.append(jax.core.ShapedArray(shape, dtype))
            zero_outs.append(np.zeros((N_CORES * shape[0], *shape[1:]), dtype))
    n_params = len(in_names)
    all_names = tuple(in_names + out_names)

    def _body(*args):
        operands = list(args)
        if partition_name is not None:
            operands.append(bass2jax.partition_id_tensor())
        outs = bass2jax._bass_exec_p.bind(
            *operands,
            out_avals=tuple(out_avals),
            in_names=all_names + ((partition_name,) if partition_name else ()),
            out_names=tuple(out_names),
            lowering_input_output_aliases=(),
            sim_require_finite=True,
            sim_require_nnan=True,
            nc=nc,
        )
        return tuple(outs)

    devices = jax.devices()[:N_CORES]
    assert len(devices) == N_CORES
    mesh = Mesh(np.asarray(devices), ("core",))
    nspec = NamedSharding(mesh, PartitionSpec("core"))
    in_specs = (PartitionSpec("core"),) * (n_params + len(out_names))
    out_specs = (PartitionSpec("core"),) * len(out_names)
    fn = jax.jit(
        shard_map(_body, mesh=mesh, in_specs=in_specs, out_specs=out_specs,
                  check_rep=False),
        keep_unused=True,
    )
    zeros_dev = [jax.device_put(z, nspec) for z in zero_outs]

    def run(in_maps):
        concat = [
            np.concatenate([np.asarray(in_maps[c][nm]) for c in range(N_CORES)],
                           axis=0)
            for nm in in_names
        ]
        outs = fn(*concat, *zeros_dev)
        return [
            {nm: np.asarray(outs[i]).reshape(N_CORES, *out_avals[i].shape)[c]
             for i, nm in enumerate(out_names)}
            for c in range(N_CORES)
        ]

    return run


def _prep_inputs(inputs):
    """Host-side marshaling: quantize+shard context, gather positions,
    fp16 blob pack."""
    f = np.float32
    h = np.float16
    ctx_f = np.asarray(inputs["context_feat"], f)
    ctx_c = np.asarray(inputs["context_coord"], f)

    # int8 per-row quantized context records: 128 x i8 | f16 scale | pad
    rowmax = np.abs(ctx_f).max(axis=1)
    rsc = np.maximum(rowmax, 1e-12) / 127.0                    # [N] f32
    q8 = np.clip(np.round(ctx_f / rsc[:, None]), -127, 127).astype(np.int8)
    tab = np.zeros((N, ROWB), np.uint8)
    tab[:, :C] = q8.view(np.uint8)
    tab[:, C:C + 2] = rsc.astype(h).reshape(N, 1).view(np.uint8)

    s = lambda g_: (np.asarray(g_, f) / np.sqrt(np.float32(1.0 + EPS_BN)))
    Wq = np.asarray(inputs["Wq"], f); Wk = np.asarray(inputs["Wk"], f)
    Wv = np.asarray(inputs["Wv"], f)
    Wp1 = np.asarray(inputs["Wp1"], f); Wp2 = np.asarray(inputs["Wp2"], f)
    Ww1 = np.asarray(inputs["Ww1"], f); Ww2 = np.asarray(inputs["Ww2"], f)

    sq = s(inputs["gq"]); bq = sq * inputs["bq"] + np.asarray(inputs["betaq"], f)
    sk = s(inputs["gk"]); bk = sk * inputs["bk"] + np.asarray(inputs["betak"], f)
    sp1 = s(inputs["gp1"])
    bp1 = sp1 * inputs["bp1"] + np.asarray(inputs["betap1"], f)
    bv = np.asarray(inputs["bv"], f) + np.asarray(inputs["bp2"], f)  # val bias
    # stacked bn for w1: row 8i+g ; fold bp2@Ww1 into bias
    sw1_g = s(inputs["gw1"])                                   # [G]
    bw1_g = (sw1_g * (np.asarray(inputs["bw1"], f)
                      + np.asarray(inputs["bp2"], f) @ Ww1)
             + np.asarray(inputs["betaw1"], f))                # [G]
    sw1 = np.tile(sw1_g, 16).astype(f)
    bw1 = np.tile(bw1_g, 16).astype(f)

    P2W1 = (Wp2 @ Ww1).astype(f)                               # [C, G]

    knn = np.asarray(inputs["knn_indexes"])
    knn = np.where(knn < 0, 0, knn).astype(np.int32)
    qf = np.asarray(inputs["query_feat"], f)
    qc = np.asarray(inputs["query_coord"], f)

    q_full = np.maximum(sq * (qf @ Wq) + bq, 0.0)          # [M, C]
    nqwT = (-(q_full @ Ww1)).T.astype(f)                   # [G, M]
    fixed = {"Wk": Wk, "Wv": Wv, "Wp2": Wp2, "Ww2s": Ww2,
             "Ww1": Ww1, "P2W1": P2W1, "Wp1": Wp1}
    fixed16 = {nm: np.asarray(v, f).astype(h).ravel() for nm, v in fixed.items()}

    in_maps = []
    for c in range(N_CORES):
        sl = slice(c * M_LOC, (c + 1) * M_LOC)
        idx = knn[sl].reshape(-1)                        # [R_LOC] m*16+k order
        knn_t = idx.reshape(R_LOC // C, C).T.copy()      # [128, R_LOC/128] i32
        pos = (ctx_c[idx] - np.repeat(qc[sl], K, axis=0))        # [R_LOC, 3]
        psc = np.float32(max(np.abs(pos).max(), 1e-12) / 127.0)
        p8 = np.round(pos / psc).astype(np.int8).T               # [3, R_LOC]
        # fold the pos dequant scale into the positional-BN scale
        scal = np.stack([sq, bq, sk, bk, bv, sp1 * psc, bp1, sw1, bw1],
                        axis=1)                                  # [C, 9]
        blob = np.empty(_NTOT, h)
        pieces = dict(fixed16)
        pieces["ctxtab"] = np.ascontiguousarray(
            tab[c * N_LOC:(c + 1) * N_LOC]).reshape(-1).view(h)
        pieces["nqwT"] = nqwT[:, sl].astype(h).ravel()
        pieces["knn16"] = knn_t.view(h).ravel()
        pieces["posT8"] = np.ascontiguousarray(p8).reshape(-1).view(h)
        pieces["scal"] = scal.astype(h).ravel()
        for nm, (p_, c_) in _LAYOUT:
            off = _OFFS[nm]
            blob[off:off + p_ * c_] = pieces[nm]
        in_maps.append({"blob": blob})
    return in_maps


def _get():
    global _compiled
    if _compiled is None:
        nc = _build()
        _compiled = (nc, _make_runner(nc))
    return _compiled


def _decode(res):
    """u8 per-channel quantized device output -> full [M, C] fp32."""
    outs = []
    for c in range(N_CORES):
        a = res[c]["out"]                                  # [C, M_LOC+4] u8
        m_ = np.ascontiguousarray(a[:, M_LOC:M_LOC + 4]).view(np.float32)
        y = a[:, :M_LOC].astype(np.float32)
        outs.append(((y - 128.0) * (m_ / 126.99)).T)
    return np.ascontiguousarray(np.concatenate(outs, axis=0).astype(np.float32))


def kernel(**inputs):
    nc, run = _get()
    in_maps = _prep_inputs(inputs)
    return _decode(run(in_maps))


# revision 25
# speedup vs baseline: 1.0823x; 1.0823x over previous
"""KNN grouped-vector-attention pool kernel for 8 Trainium2 NeuronCores.

Strategy: shard queries M=16384 across 8 cores (2048 each). The context
feature table is sharded across cores and reassembled on device with an HBM
AllGather; each core then resolves its own KNN gathers locally via indirect
DMA and XBAR DMA-transposes into channel-major layout.

Interconnect to the device is the bottleneck (slow tunneled PJRT link), so
the wire format is aggressively compressed:
  * only context rows actually referenced by knn_indexes ship (~86.5% of N
    for random indexes; knn renumbered on host), as 130-byte records:
    128 x int8 (per-row absmax quantized) + the fp16 row scale. Dequant
    happens on device right after each 128-row gather (ACT engine,
    per-partition scale).
    The AllGather moves the records typed as u16: the collective datapath
    routes elements through f32, so u8/u16 round-trip exactly while f16
    wires flush denormals and i32 wires lose low mantissa bits.
  * relative positions ship as int8 with a per-core global scale folded
    into the positional-BN scale scalar.
  * the output returns as uint8 (per-channel absmax quantized, bias +128)
    with the per-channel fp32 scale appended to the row.
  * the query projection is rank-reduced on the host: q only enters the
    logits as q@Ww1 [M,8], so an [8,M] fp16 stripe uploads instead of the
    [128,M] query block.
  * Ww2 uploads as a single 8x8 tile; its 16-block block-diagonal expansion
    is assembled on device. Sel / E / Ww1s / WpW1s selector matrices are
    synthesized on device from tiny seeds.
All per-core inputs pack into one contiguous fp16 blob (~2.4MB) so the
host->device path pays a single transfer per core. The dispatch path is a
custom PJRT runner that keeps the donated-zero output buffers device-
resident, avoiding run_bass_kernel_spmd's per-call zero upload. All matmuls
run fp16 x fp16 with fp32 PSUM accumulation.
"""
import sys
sys.path.insert(0, '/opt/trn_rl_repo')
import numpy as np

N_CORES = 8
M, N, K, C, G = 16384, 131072, 16, 128, 8
M_LOC = M // N_CORES          # 2048 queries per core
R_LOC = M_LOC * K             # 32768 gathered rows per core
N_LOC = N // N_CORES          # 16384 context rows uploaded per core
CHUNK = 512                   # rows per compute chunk (one PSUM bank)
GROUP = 16 * CHUNK            # 8192 rows per stacked group
N_GROUPS = R_LOC // GROUP     # 4
ROWB = 130                    # context record: 128 int8 + fp16 row scale
EPS_BN = 1e-5

# The table holds only the context rows actually referenced by knn_indexes
# (~86.5% of N for random indexes); knn is renumbered on the host. The cap
# is fixed at first call (shapes are baked into the compiled kernel).
_CAP = None
_LAYOUT = None
_OFFS = None
_NTOT = None


def _set_layout(cap):
    global _CAP, _LAYOUT, _OFFS, _NTOT
    if _CAP is not None:
        assert cap <= _CAP, f"table cap {cap} exceeds compiled {_CAP}"
        return
    _CAP = cap
    # blob layout: (name, (partitions, cols)) packed row-major, fp16 units
    _LAYOUT = [
        ("ctxtab", (C, cap // N_CORES * ROWB // 2 // C)),  # u8 records
        ("nqwT8", (G, M_LOC // 2)),  # -(relu(bn(qf@Wq)) @ Ww1).T int8
        ("Wk", (C, C)), ("Wv", (C, C)), ("Wp2", (C, C)),
        ("Ww2s", (G, G)),
        ("Ww1", (C, G)), ("P2W1", (C, G)),
        ("scal", (C, 10)),           # sq,bq,sk,bk,bv,sp1,bp1,sw1,bw1,snq
        ("knn16", (C, R_LOC // C * 2)),  # [128,256] i32 KNN blocks, fp16 bits
        ("posT8", (3, R_LOC // 2)),  # [3, R_LOC] int8, fp16 bits
        ("Wp1", (3, C)),
    ]
    _OFFS = {}
    tot = 0
    for nm, (p, c) in _LAYOUT:
        _OFFS[nm] = tot
        tot += p * c
    _NTOT = tot


_compiled = None


def _build():
    from concourse import bacc, bass, mybir
    import concourse.tile as tile

    f32 = mybir.dt.float32
    f16 = mybir.dt.float16
    i32 = mybir.dt.int32
    i8 = mybir.dt.int8
    u8 = mybir.dt.uint8
    u16 = mybir.dt.uint16
    AF = mybir.ActivationFunctionType
    OP = mybir.AluOpType

    nc = bacc.Bacc("TRN2", target_bir_lowering=False, debug=False,
                   num_devices=N_CORES)

    blob = nc.dram_tensor("blob", (_NTOT,), f16, kind="ExternalInput").ap()
    out_d = nc.dram_tensor("out", (C, M_LOC + 4), u8,
                           kind="ExternalOutput").ap()

    def view(nm):
        p, c = dict(_LAYOUT)[nm]
        off = _OFFS[nm]
        return blob[off:off + p * c].rearrange("(p c) -> p c", p=p)

    from contextlib import ExitStack
    est = ExitStack()
    with tile.TileContext(nc) as tc, est:
        dpool = est.enter_context(tc.tile_pool(name="dram", bufs=1, space="DRAM"))
        cpool = est.enter_context(tc.tile_pool(name="const", bufs=1))
        gtpool = est.enter_context(tc.tile_pool(name="gt", bufs=8))
        g16pool = est.enter_context(tc.tile_pool(name="g16", bufs=8))
        scpool = est.enter_context(tc.tile_pool(name="gsc", bufs=8))
        gpool = est.enter_context(tc.tile_pool(name="gath", bufs=2))
        p8pool = est.enter_context(tc.tile_pool(name="p8st", bufs=2))
        vpool = est.enter_context(tc.tile_pool(name="valp", bufs=2))
        spool = est.enter_context(tc.tile_pool(name="work", bufs=2))
        opool = est.enter_context(tc.tile_pool(name="outp", bufs=1))
        ps = {}
        for nm, nb in [("kp", 2), ("px", 2), ("vp", 2), ("stk", 1), ("wr", 1)]:
            ps[nm] = est.enter_context(tc.tile_pool(name=nm, bufs=nb, space="PSUM"))

        # ---- AllGather the context record table in HBM ----------------
        # wire dtype MUST be an int type <= 16 bits: the collective datapath
        # routes elements through f32 (u16/u8 round-trip exactly; f16
        # denormals get flushed and i32 loses low mantissa bits)
        tp, tc_ = dict(_LAYOUT)["ctxtab"]
        ib = dpool.tile([tp, tc_], u16, tag="ib", name="ib")
        ob = dpool.tile([tp, tc_ * N_CORES], u16, tag="ob", name="ob",
                        addr_space="Shared")
        nc.gpsimd.dma_start(ib[:], view("ctxtab").bitcast(u16))
        nc.gpsimd.collective_compute(
            "AllGather", OP.bypass,
            replica_groups=[list(range(N_CORES))],
            ins=[ib.opt()], outs=[ob.opt()])
        # reinterpret the gathered flat buffer as [N, 132] u8 records
        ctx2d = ob[:].bitcast(u8).rearrange("p (r c) -> (p r) c", c=ROWB)

        # ---- constants into SBUF -------------------------------------
        ct = {}
        for nm in ("nqwT8", "Wk", "Wv", "Wp2", "Ww2s", "Ww1", "P2W1",
                   "scal", "knn16", "Wp1"):
            p, c = dict(_LAYOUT)[nm]
            ct[nm] = cpool.tile([p, c], f16, tag=f"c_{nm}", name=f"c_{nm}")
            nc.sync.dma_start(out=ct[nm][:], in_=view(nm))
        knn32 = ct["knn16"][:].bitcast(i32)          # [128, R_LOC/128] i32
        # fp16 scalars -> f32 working copy; per-scalar column APs
        scal32 = cpool.tile([C, 10], f32, tag="c_scal32", name="c_scal32")
        nc.vector.tensor_copy(out=scal32[:], in_=ct["scal"][:])
        for j, nm in enumerate(("sq", "bq", "sk", "bk", "bv", "sp1", "bp1",
                                "sw1", "bw1")):
            ct[nm] = scal32[:, j:j + 1]
        # dequantize the int8 q stripe: nq16 = nq8 * snq
        nq16 = cpool.tile([G, M_LOC], f16, tag="c_nq16", name="c_nq16")
        nc.vector.tensor_copy(out=nq16[:], in_=ct["nqwT8"][:].bitcast(i8))
        nc.vector.tensor_scalar(out=nq16[:], in0=nq16[:],
                                scalar1=scal32[0:G, 9:10], scalar2=None,
                                op0=OP.mult)

        # W2bd: 16-block block-diagonal expansion of Ww2 (8x8)
        w2bd = cpool.tile([C, C], f16, tag="c_w2bd", name="c_w2bd")
        nc.gpsimd.memset(w2bd[:], 0.0)
        for i in range(16):
            nc.sync.dma_start(out=w2bd[8 * i:8 * i + 8, 8 * i:8 * i + 8],
                              in_=ct["Ww2s"][:])

        # ---- synthesize Sel / Ww1s / WpW1s on device -----------------
        # Sel[p, j] = 1 iff j // 16 == p  (i.e. 0 <= j - 16p <= 15)
        sel = cpool.tile([C, 16 * C], f16, tag="c_sel", name="c_sel")
        nc.gpsimd.memset(sel[:], 1.0)
        nc.gpsimd.affine_select(out=sel[:], in_=sel[:], compare_op=OP.is_ge,
                                fill=0.0, base=0, pattern=[[1, 16 * C]],
                                channel_multiplier=-16)
        nc.gpsimd.affine_select(out=sel[:], in_=sel[:], compare_op=OP.is_gt,
                                fill=0.0, base=16, pattern=[[-1, 16 * C]],
                                channel_multiplier=16)
        # E block i maps the 8 q-logit rows onto stacked partitions 8i+g
        etile = cpool.tile([G, 16 * C], f16, tag="c_e", name="c_e")
        nc.gpsimd.memset(etile[:], 0.0)
        nc.gpsimd.affine_select(out=etile[:], in_=etile[:],
                                compare_op=OP.not_equal, fill=1.0, base=0,
                                pattern=[[-8, 16], [1, C]],
                                channel_multiplier=-1)
        # Ww1s block i holds Ww1 at cols i*C + 8i .. +8 (rest zero)
        ww1s = cpool.tile([C, 16 * C], f16, tag="c_ww1s", name="c_ww1s")
        wpw1s = cpool.tile([C, 16 * C], f16, tag="c_wpw1s", name="c_wpw1s")
        nc.gpsimd.memset(ww1s[:], 0.0)
        nc.gpsimd.memset(wpw1s[:], 0.0)
        for i in range(16):
            c0 = i * C + 8 * i
            nc.vector.tensor_copy(out=ww1s[:, c0:c0 + 8], in_=ct["Ww1"][:])
            nc.vector.tensor_copy(out=wpw1s[:, c0:c0 + 8], in_=ct["P2W1"][:])

        outT = opool.tile([C, M_LOC], f32)

        for g in range(N_GROUPS):
            fT = gpool.tile([C, GROUP], f16, tag="fT")
            # gather + dequant + transpose this group's 8192 neighbor rows
            for blk in range(GROUP // C):
                gcol = g * (GROUP // C) + blk
                gt = gtpool.tile([C, ROWB], u8, tag="gt")
                nc.gpsimd.indirect_dma_start(
                    out=gt[:], out_offset=None,
                    in_=ctx2d,
                    in_offset=bass.IndirectOffsetOnAxis(
                        ap=knn32[:, gcol:gcol + 1], axis=0))
                sc32 = scpool.tile([C, 1], f32, tag="gsc")
                nc.vector.tensor_copy(out=sc32[:],
                                      in_=gt[:, 128:130].bitcast(f16))
                gt16 = g16pool.tile([C, C], f16, tag="g16")
                nc.scalar.activation(out=gt16[:], in_=gt[:, 0:128].bitcast(i8),
                                     func=AF.Identity, bias=0.0,
                                     scale=sc32[:])
                eng = nc.sync if blk % 2 == 0 else nc.scalar
                eng.dma_start_transpose(
                    out=fT[:, blk * C:(blk + 1) * C], in_=gt16[:])
            pT8 = p8pool.tile([3, GROUP // 2], f16, tag="pT8")
            nc.sync.dma_start(
                out=pT8[:],
                in_=view("posT8")[:, g * (GROUP // 2):(g + 1) * (GROUP // 2)])
            pT = gpool.tile([3, GROUP], f16, tag="pT")
            nc.vector.tensor_copy(out=pT[:], in_=pT8[:].bitcast(i8))
            valT = vpool.tile([C, GROUP], f32, tag="valp")
            stacked_ps = ps["stk"].tile([C, CHUNK], f32, tag="stk_t", name="stacked_ps")
            # -------- phase A: per chunk of 512 gathered rows ---------
            for i in range(16):
                ch = g * 16 + i              # global chunk id
                q0 = ch * 32                 # first query of chunk
                ctx = fT[:, i * CHUNK:(i + 1) * CHUNK]
                pos = pT[:, i * CHUNK:(i + 1) * CHUNK]
                # key = relu(bn(Wk.T @ ctx))
                k_ps = ps["kp"].tile([C, CHUNK], f32, tag="kp_t", name="k_ps")
                nc.tensor.matmul(out=k_ps[:], lhsT=ct["Wk"][:], rhs=ctx,
                                 start=True, stop=True)
                keyT = spool.tile([C, CHUNK], f16, tag="keyT")
                nc.scalar.activation(out=keyT[:], in_=k_ps[:], func=AF.Relu,
                                     bias=ct["bk"], scale=ct["sk"])
                # pebx = relu(bn(Wp1.T @ pos))
                pebx_ps = ps["px"].tile([C, CHUNK], f32, tag="px_t", name="pebx_ps")
                nc.tensor.matmul(out=pebx_ps[:], lhsT=ct["Wp1"][:], rhs=pos,
                                 start=True, stop=True)
                pebxT = spool.tile([C, CHUNK], f16, tag="pebxT")
                nc.scalar.activation(out=pebxT[:], in_=pebx_ps[:], func=AF.Relu,
                                     bias=ct["bp1"], scale=ct["sp1"])
                # val = Wv.T @ ctx + Wp2.T @ pebx (+ bv + bp2 via bias)
                v_ps = ps["vp"].tile([C, CHUNK], f32, tag="vp_t", name="v_ps")
                nc.tensor.matmul(out=v_ps[:], lhsT=ct["Wv"][:], rhs=ctx,
                                 start=True, stop=False)
                nc.tensor.matmul(out=v_ps[:], lhsT=ct["Wp2"][:], rhs=pebxT[:],
                                 start=False, stop=True)
                nc.scalar.activation(out=valT[:, i * CHUNK:(i + 1) * CHUNK],
                                     in_=v_ps[:], func=AF.Identity,
                                     bias=ct["bv"], scale=1.0)
                # w1 logits, stacked: Ww1.T @ (key - q + peb) with
                # peb folded via WpW1s = Wp2 @ Ww1s and -q via nqT
                q_rep = nq16[:, q0:q0 + 32].unsqueeze(2) \
                    .to_broadcast([G, 32, K])
                nc.tensor.matmul(out=stacked_ps[:],
                                 lhsT=ww1s[:, i * C:(i + 1) * C],
                                 rhs=keyT[:], start=(i == 0), stop=False,
                                 skip_group_check=True)
                nc.tensor.matmul(out=stacked_ps[:],
                                 lhsT=wpw1s[:, i * C:(i + 1) * C],
                                 rhs=pebxT[:], start=False, stop=False,
                                 skip_group_check=True)
                nc.tensor.matmul(out=stacked_ps[:],
                                 lhsT=etile[:, i * C:(i + 1) * C],
                                 rhs=q_rep, start=False, stop=(i == 15),
                                 skip_group_check=True)
            # -------- group tail: bn/relu, mm2, softmax ---------------
            stk_bn = spool.tile([C, CHUNK], f16, tag="stkbn")
            nc.scalar.activation(out=stk_bn[:], in_=stacked_ps[:], func=AF.Relu,
                                 bias=ct["bw1"], scale=ct["sw1"])
            w2_ps = ps["px"].tile([C, CHUNK], f32, tag="px_t", name="w2_ps")
            nc.tensor.matmul(out=w2_ps[:], lhsT=w2bd[:], rhs=stk_bn[:],
                             start=True, stop=True)
            mx = spool.tile([C, 32], f32, tag="mx")
            nc.vector.tensor_reduce(
                out=mx[:], in_=w2_ps[:].rearrange("p (m k) -> p m k", k=K),
                axis=mybir.AxisListType.X, op=OP.max)
            sm = spool.tile([C, CHUNK], f32, tag="sm")
            nc.vector.tensor_tensor(
                out=sm[:].rearrange("p (m k) -> p m k", k=K),
                in0=w2_ps[:].rearrange("p (m k) -> p m k", k=K),
                in1=mx[:].unsqueeze(2).to_broadcast([C, 32, K]),
                op=OP.subtract)
            e_t = spool.tile([C, CHUNK], f32, tag="e")
            nc.scalar.activation(out=e_t[:], in_=sm[:], func=AF.Exp)
            s_t = spool.tile([C, 32], f32, tag="s")
            nc.vector.tensor_reduce(
                out=s_t[:], in_=e_t[:].rearrange("p (m k) -> p m k", k=K),
                axis=mybir.AxisListType.X, op=OP.add)
            rinv = spool.tile([C, 32], f32, tag="rinv")
            nc.vector.reciprocal(out=rinv[:], in_=s_t[:])
            wf32 = spool.tile([C, CHUNK], f32, tag="wf32")
            nc.vector.tensor_tensor(
                out=wf32[:].rearrange("p (m k) -> p m k", k=K),
                in0=e_t[:].rearrange("p (m k) -> p m k", k=K),
                in1=rinv[:].unsqueeze(2).to_broadcast([C, 32, K]),
                op=OP.mult)
            wfin = spool.tile([C, CHUNK], f16, tag="wfin")
            nc.scalar.activation(out=wfin[:], in_=wf32[:], func=AF.Identity)
            # -------- phase B: weighted sum per chunk -----------------
            for i in range(16):
                ch = g * 16 + i
                wrep_ps = ps["wr"].tile([C, CHUNK], f32, tag="wr_t", name="wrep_ps")
                nc.tensor.matmul(out=wrep_ps[:],
                                 lhsT=sel[:, i * C:(i + 1) * C],
                                 rhs=wfin[:], start=True, stop=True)
                prod = spool.tile([C, CHUNK], f32, tag="prod")
                nc.vector.tensor_tensor(out=prod[:],
                                        in0=valT[:, i * CHUNK:(i + 1) * CHUNK],
                                        in1=wrep_ps[:], op=OP.mult)
                nc.vector.tensor_reduce(
                    out=outT[:, ch * 32:(ch + 1) * 32],
                    in_=prod[:].rearrange("p (m k) -> p m k", k=K),
                    axis=mybir.AxisListType.X, op=OP.add)

        # -------- output: per-channel u8 quantization -----------------
        m_t = opool.tile([C, 1], f32, tag="omax", name="omax")
        mn_t = opool.tile([C, 1], f32, tag="omin", name="omin")
        nc.vector.tensor_reduce(out=m_t[:], in_=outT[:],
                                axis=mybir.AxisListType.X, op=OP.max)
        nc.vector.tensor_reduce(out=mn_t[:], in_=outT[:],
                                axis=mybir.AxisListType.X, op=OP.min)
        nc.vector.tensor_scalar_mul(out=mn_t[:], in0=mn_t[:], scalar1=-1.0)
        nc.vector.tensor_tensor(out=m_t[:], in0=m_t[:], in1=mn_t[:],
                                op=OP.max)
        nc.vector.tensor_scalar_max(out=m_t[:], in0=m_t[:], scalar1=1e-20)
        rs = opool.tile([C, 1], f32, tag="orsc", name="orsc")
        nc.vector.reciprocal(out=rs[:], in_=m_t[:])
        # 126.99 (not 127) so the row-max element lands strictly below
        # 255.5 after the +128.5 shift even if reciprocal rounds up --
        # keeps the u8 convert away from any wrap/saturate edge.
        nc.vector.tensor_scalar_mul(out=rs[:], in0=rs[:], scalar1=126.99)
        y8 = opool.tile([C, M_LOC], u8, tag="oy8", name="oy8")
        nc.vector.tensor_scalar(out=y8[:], in0=outT[:], scalar1=rs[:],
                                scalar2=128.5, op0=OP.mult, op1=OP.add)
        nc.sync.dma_start(out=out_d[:, 0:M_LOC], in_=y8[:])
        nc.sync.dma_start(out=out_d[:, M_LOC:M_LOC + 4].bitcast(f32),
                          in_=m_t[:])

    nc.compile()
    return nc


def _make_runner(nc):
    """PJRT dispatch for the compiled Bass module, mirroring
    bass2jax.run_bass_via_pjrt but keeping the (ignored) donated-zero
    output operands device-resident so each call uploads only the blob."""
    import jax
    from jax.sharding import Mesh, PartitionSpec, NamedSharding
    from jax.experimental.shard_map import shard_map
    from concourse import bass2jax, mybir

    bass2jax.install_neuronx_cc_hook()
    assert nc.dbg_addr is None or not nc.dbg_callbacks

    partition_name = (nc.partition_id_tensor.name
                      if nc.partition_id_tensor else None)
    in_names, in_shapes = [], {}
    out_names, out_avals, zero_outs = [], [], []
    for alloc in nc.m.functions[0].allocations:
        if not isinstance(alloc, mybir.MemoryLocationSet):
            continue
        name = alloc.memorylocations[0].name
        if alloc.kind == "ExternalInput":
            if name != partition_name:
                in_names.append(name)
                in_shapes[name] = (tuple(alloc.tensor_shape),
                                   mybir.dt.np(alloc.dtype))
        elif alloc.kind == "ExternalOutput":
            shape = tuple(alloc.tensor_shape)
            dtype = mybir.dt.np(alloc.dtype)
            out_names.append(name)
            out_avals.append(jax.core.ShapedArray(shape, dtype))
            zero_outs.append(np.zeros((N_CORES * shape[0], *shape[1:]), dtype))
    n_params = len(in_names)
    all_names = tuple(in_names + out_names)

    def _body(*args):
        operands = list(args)
        if partition_name is not None:
            operands.append(bass2jax.partition_id_tensor())
        outs = bass2jax._bass_exec_p.bind(
            *operands,
            out_avals=tuple(out_avals),
            in_names=all_names + ((partition_name,) if partition_name else ()),
            out_names=tuple(out_names),
            lowering_input_output_aliases=(),
            sim_require_finite=True,
            sim_require_nnan=True,
            nc=nc,
        )
        return tuple(outs)

    devices = jax.devices()[:N_CORES]
    assert len(devices) == N_CORES
    mesh = Mesh(np.asarray(devices), ("core",))
    nspec = NamedSharding(mesh, PartitionSpec("core"))
    in_specs = (PartitionSpec("core"),) * (n_params + len(out_names))
    out_specs = (PartitionSpec("core"),) * len(out_names)
    fn = jax.jit(
        shard_map(_body, mesh=mesh, in_specs=in_specs, out_specs=out_specs,
                  check_rep=False),
        keep_unused=True,
    )
    zeros_dev = [jax.device_put(z, nspec) for z in zero_outs]

    def run(in_maps):
        concat = [
            np.concatenate([np.asarray(in_maps[c][nm]) for c in range(N_CORES)],
                           axis=0)
            for nm in in_names
        ]
        outs = fn(*concat, *zeros_dev)
        return [
            {nm: np.asarray(outs[i]).reshape(N_CORES, *out_avals[i].shape)[c]
             for i, nm in enumerate(out_names)}
            for c in range(N_CORES)
        ]

    return run


def _prep_inputs(inputs):
    """Host-side marshaling: quantize+shard context, gather positions,
    fp16 blob pack."""
    f = np.float32
    h = np.float16
    ctx_f = np.asarray(inputs["context_feat"], f)
    ctx_c = np.asarray(inputs["context_coord"], f)

    knn = np.asarray(inputs["knn_indexes"])
    knn = np.where(knn < 0, 0, knn).astype(np.int32)
    # compact the table to referenced rows only, renumbering knn
    used = np.unique(knn)                                      # sorted
    cap = (len(used) + 512 + 8191) // 8192 * 8192
    _set_layout(cap)
    cap = _CAP
    knn_r = np.searchsorted(used, knn).astype(np.int32)        # [M, K]

    # int8 per-row quantized context records: 128 x i8 | f16 row scale
    rowmax = np.abs(ctx_f).max(axis=1)
    rsc = np.maximum(rowmax, 1e-12) / 127.0                    # [N] f32
    q8 = np.clip(np.round(ctx_f / rsc[:, None]), -127, 127).astype(np.int8)
    tab = np.zeros((cap, ROWB), np.uint8)
    tab[:len(used), :C] = q8[used].view(np.uint8)
    tab[:len(used), C:C + 2] = rsc[used].astype(h).reshape(-1, 1) \
        .view(np.uint8)

    s = lambda g_: (np.asarray(g_, f) / np.sqrt(np.float32(1.0 + EPS_BN)))
    Wq = np.asarray(inputs["Wq"], f); Wk = np.asarray(inputs["Wk"], f)
    Wv = np.asarray(inputs["Wv"], f)
    Wp1 = np.asarray(inputs["Wp1"], f); Wp2 = np.asarray(inputs["Wp2"], f)
    Ww1 = np.asarray(inputs["Ww1"], f); Ww2 = np.asarray(inputs["Ww2"], f)

    sq = s(inputs["gq"]); bq = sq * inputs["bq"] + np.asarray(inputs["betaq"], f)
    sk = s(inputs["gk"]); bk = sk * inputs["bk"] + np.asarray(inputs["betak"], f)
    sp1 = s(inputs["gp1"])
    bp1 = sp1 * inputs["bp1"] + np.asarray(inputs["betap1"], f)
    bv = np.asarray(inputs["bv"], f) + np.asarray(inputs["bp2"], f)  # val bias
    # stacked bn for w1: row 8i+g ; fold bp2@Ww1 into bias
    sw1_g = s(inputs["gw1"])                                   # [G]
    bw1_g = (sw1_g * (np.asarray(inputs["bw1"], f)
                      + np.asarray(inputs["bp2"], f) @ Ww1)
             + np.asarray(inputs["betaw1"], f))                # [G]
    sw1 = np.tile(sw1_g, 16).astype(f)
    bw1 = np.tile(bw1_g, 16).astype(f)

    P2W1 = (Wp2 @ Ww1).astype(f)                               # [C, G]

    qf = np.asarray(inputs["query_feat"], f)
    qc = np.asarray(inputs["query_coord"], f)

    q_full = np.maximum(sq * (qf @ Wq) + bq, 0.0)          # [M, C]
    nqwT = (-(q_full @ Ww1)).T.astype(f)                   # [G, M]
    snq = np.float32(max(np.abs(nqwT).max(), 1e-12) / 127.0)
    nq8 = np.round(nqwT / snq).astype(np.int8)             # [G, M]
    fixed = {"Wk": Wk, "Wv": Wv, "Wp2": Wp2, "Ww2s": Ww2,
             "Ww1": Ww1, "P2W1": P2W1, "Wp1": Wp1}
    fixed16 = {nm: np.asarray(v, f).astype(h).ravel() for nm, v in fixed.items()}

    tab_loc = cap // N_CORES
    in_maps = []
    for c in range(N_CORES):
        sl = slice(c * M_LOC, (c + 1) * M_LOC)
        idx = knn[sl].reshape(-1)          # original ids, for coord gather
        knn_t = knn_r[sl].reshape(R_LOC // C, C).T.copy()  # renumbered, i32
        pos = (ctx_c[idx] - np.repeat(qc[sl], K, axis=0))        # [R_LOC, 3]
        psc = np.float32(max(np.abs(pos).max(), 1e-12) / 127.0)
        p8 = np.round(pos / psc).astype(np.int8).T               # [3, R_LOC]
        # fold the pos dequant scale into the positional-BN scale
        scal = np.stack([sq, bq, sk, bk, bv, sp1 * psc, bp1, sw1, bw1,
                         np.full(C, snq, f)], axis=1)            # [C, 10]
        blob = np.empty(_NTOT, h)
        pieces = dict(fixed16)
        pieces["ctxtab"] = np.ascontiguousarray(
            tab[c * tab_loc:(c + 1) * tab_loc]).reshape(-1).view(h)
        pieces["nqwT8"] = np.ascontiguousarray(nq8[:, sl]).reshape(-1).view(h)
        pieces["knn16"] = knn_t.view(h).ravel()
        pieces["posT8"] = np.ascontiguousarray(p8).reshape(-1).view(h)
        pieces["scal"] = scal.astype(h).ravel()
        for nm, (p_, c_) in _LAYOUT:
            off = _OFFS[nm]
            blob[off:off + p_ * c_] = pieces[nm]
        in_maps.append({"blob": blob})
    return in_maps


def _get():
    global _compiled
    if _compiled is None:
        nc = _build()
        _compiled = (nc, _make_runner(nc))
    return _compiled


def _decode(res):
    """u8 per-channel quantized device output -> full [M, C] fp32."""
    outs = []
    for c in range(N_CORES):
        a = res[c]["out"]                                  # [C, M_LOC+4] u8
        m_ = np.ascontiguousarray(a[:, M_LOC:M_LOC + 4]).view(np.float32)
        y = a[:, :M_LOC].astype(np.float32)
        outs.append(((y - 128.0) * (m_ / 126.99)).T)
    return np.ascontiguousarray(np.concatenate(outs, axis=0).astype(np.float32))


def kernel(**inputs):
    in_maps = _prep_inputs(inputs)   # sets the table cap before first build
    nc, run = _get()
    return _decode(run(in_maps))


# revision 26
# speedup vs baseline: 1.1459x; 1.0588x over previous
"""KNN grouped-vector-attention pool kernel for 8 Trainium2 NeuronCores.

Strategy: shard queries M=16384 across 8 cores (2048 each). The context
feature table is sharded across cores and reassembled on device with an HBM
AllGather; each core then resolves its own KNN gathers locally via indirect
DMA and XBAR DMA-transposes into channel-major layout.

Interconnect to the device is the bottleneck (slow tunneled PJRT link), so
the wire format is aggressively compressed:
  * only context rows actually referenced by knn_indexes ship (~86.5% of N
    for random indexes; knn renumbered on host), as 130-byte records:
    128 x int8 (per-row absmax quantized) + the fp16 row scale. Dequant
    happens on device right after each 128-row gather (ACT engine,
    per-partition scale).
    The AllGather moves the records typed as u16: the collective datapath
    routes elements through f32, so u8/u16 round-trip exactly while f16
    wires flush denormals and i32 wires lose low mantissa bits.
  * relative positions ship as int8 with a per-core global scale folded
    into the positional-BN scale scalar.
  * the output returns as uint8 (per-channel absmax quantized, bias +128)
    with the per-channel fp32 scale appended to the row.
  * the query projection is rank-reduced on the host: q only enters the
    logits as q@Ww1 [M,8], so an [8,M] fp16 stripe uploads instead of the
    [128,M] query block.
  * Ww2 uploads as a single 8x8 tile; its 16-block block-diagonal expansion
    is assembled on device. Sel / E / Ww1s / WpW1s selector matrices are
    synthesized on device from tiny seeds.
All per-core inputs pack into one contiguous fp16 blob (~2.4MB) so the
host->device path pays a single transfer per core. The dispatch path is a
custom PJRT runner that keeps the donated-zero output buffers device-
resident, avoiding run_bass_kernel_spmd's per-call zero upload. All matmuls
run fp16 x fp16 with fp32 PSUM accumulation.
"""
import sys
sys.path.insert(0, '/opt/trn_rl_repo')
import numpy as np

N_CORES = 8
M, N, K, C, G = 16384, 131072, 16, 128, 8
M_LOC = M // N_CORES          # 2048 queries per core
R_LOC = M_LOC * K             # 32768 gathered rows per core
N_LOC = N // N_CORES          # 16384 context rows uploaded per core
CHUNK = 512                   # rows per compute chunk (one PSUM bank)
GROUP = 16 * CHUNK            # 8192 rows per stacked group
N_GROUPS = R_LOC // GROUP     # 4
ROWB = 130                    # context record: 128 int8 + fp16 row scale
EPS_BN = 1e-5

# The table holds only the context rows actually referenced by knn_indexes
# (~86.5% of N for random indexes); knn is renumbered on the host. The cap
# is fixed at first call (shapes are baked into the compiled kernel).
_CAP = None
_LAYOUT = None
_OFFS = None
_NTOT = None


def _set_layout(cap):
    global _CAP, _LAYOUT, _OFFS, _NTOT
    if _CAP is not None:
        assert cap <= _CAP, f"table cap {cap} exceeds compiled {_CAP}"
        return
    _CAP = cap
    # blob layout: (name, (partitions, cols)) packed row-major, fp16 units
    _LAYOUT = [
        ("ctxtab", (C, cap // N_CORES * ROWB // 2 // C)),  # u8 records
        ("nqwT8", (G, M_LOC // 2)),  # -(relu(bn(qf@Wq)) @ Ww1).T int8
        ("Wk", (C, C)), ("Wv", (C, C)), ("Wp2", (C, C)),
        ("Ww2s", (G, G)),
        ("Ww1", (C, G)), ("P2W1", (C, G)),
        ("scal", (C, 10)),           # sq,bq,sk,bk,bv,sp1,bp1,sw1,bw1,snq
        ("knn16", (C, R_LOC // C * 2)),  # [128,256] i32 KNN blocks, fp16 bits
        ("posT8", (3, R_LOC // 2)),  # [3, R_LOC] int8, fp16 bits
        ("Wp1", (3, C)),
    ]
    _OFFS = {}
    tot = 0
    for nm, (p, c) in _LAYOUT:
        _OFFS[nm] = tot
        tot += p * c
    _NTOT = tot


_compiled = None


def _build():
    from concourse import bacc, bass, mybir
    import concourse.tile as tile

    f32 = mybir.dt.float32
    f16 = mybir.dt.float16
    i32 = mybir.dt.int32
    i8 = mybir.dt.int8
    u8 = mybir.dt.uint8
    u16 = mybir.dt.uint16
    AF = mybir.ActivationFunctionType
    OP = mybir.AluOpType

    nc = bacc.Bacc("TRN2", target_bir_lowering=False, debug=False,
                   num_devices=N_CORES)

    blob = nc.dram_tensor("blob", (_NTOT,), f16, kind="ExternalInput").ap()
    out_d = nc.dram_tensor("out", (C, M_LOC + 4), u8,
                           kind="ExternalOutput").ap()

    def view(nm):
        p, c = dict(_LAYOUT)[nm]
        off = _OFFS[nm]
        return blob[off:off + p * c].rearrange("(p c) -> p c", p=p)

    from contextlib import ExitStack
    est = ExitStack()
    with tile.TileContext(nc) as tc, est:
        dpool = est.enter_context(tc.tile_pool(name="dram", bufs=1, space="DRAM"))
        cpool = est.enter_context(tc.tile_pool(name="const", bufs=1))
        gtpool = est.enter_context(tc.tile_pool(name="gt", bufs=8))
        g16pool = est.enter_context(tc.tile_pool(name="g16", bufs=8))
        scpool = est.enter_context(tc.tile_pool(name="gsc", bufs=8))
        gpool = est.enter_context(tc.tile_pool(name="gath", bufs=2))
        p8pool = est.enter_context(tc.tile_pool(name="p8st", bufs=2))
        vpool = est.enter_context(tc.tile_pool(name="valp", bufs=2))
        spool = est.enter_context(tc.tile_pool(name="work", bufs=2))
        opool = est.enter_context(tc.tile_pool(name="outp", bufs=1))
        ps = {}
        for nm, nb in [("kp", 2), ("px", 2), ("vp", 2), ("stk", 1), ("wr", 1)]:
            ps[nm] = est.enter_context(tc.tile_pool(name=nm, bufs=nb, space="PSUM"))

        # ---- AllGather the context record table in HBM ----------------
        # wire dtype MUST be an int type <= 16 bits: the collective datapath
        # routes elements through f32 (u16/u8 round-trip exactly; f16
        # denormals get flushed and i32 loses low mantissa bits)
        tp, tc_ = dict(_LAYOUT)["ctxtab"]
        ib = dpool.tile([tp, tc_], u16, tag="ib", name="ib")
        ob = dpool.tile([tp, tc_ * N_CORES], u16, tag="ob", name="ob",
                        addr_space="Shared")
        nc.gpsimd.dma_start(ib[:], view("ctxtab").bitcast(u16))
        nc.gpsimd.collective_compute(
            "AllGather", OP.bypass,
            replica_groups=[list(range(N_CORES))],
            ins=[ib.opt()], outs=[ob.opt()])
        # reinterpret the gathered flat buffer as [N, 132] u8 records
        ctx2d = ob[:].bitcast(u8).rearrange("p (r c) -> (p r) c", c=ROWB)

        # ---- constants into SBUF -------------------------------------
        ct = {}
        for nm in ("nqwT8", "Wk", "Wv", "Wp2", "Ww2s", "Ww1", "P2W1",
                   "scal", "knn16", "Wp1"):
            p, c = dict(_LAYOUT)[nm]
            ct[nm] = cpool.tile([p, c], f16, tag=f"c_{nm}", name=f"c_{nm}")
            nc.sync.dma_start(out=ct[nm][:], in_=view(nm))
        knn32 = ct["knn16"][:].bitcast(i32)          # [128, R_LOC/128] i32
        # fp16 scalars -> f32 working copy; per-scalar column APs
        scal32 = cpool.tile([C, 10], f32, tag="c_scal32", name="c_scal32")
        nc.vector.tensor_copy(out=scal32[:], in_=ct["scal"][:])
        for j, nm in enumerate(("sq", "bq", "sk", "bk", "bv", "sp1", "bp1",
                                "sw1", "bw1")):
            ct[nm] = scal32[:, j:j + 1]
        # dequantize the int8 q stripe: nq16 = nq8 * snq
        nq16 = cpool.tile([G, M_LOC], f16, tag="c_nq16", name="c_nq16")
        nc.vector.tensor_copy(out=nq16[:], in_=ct["nqwT8"][:].bitcast(i8))
        nc.vector.tensor_scalar(out=nq16[:], in0=nq16[:],
                                scalar1=scal32[0:G, 9:10], scalar2=None,
                                op0=OP.mult)

        # W2bd: 16-block block-diagonal expansion of Ww2 (8x8)
        w2bd = cpool.tile([C, C], f16, tag="c_w2bd", name="c_w2bd")
        nc.gpsimd.memset(w2bd[:], 0.0)
        for i in range(16):
            nc.sync.dma_start(out=w2bd[8 * i:8 * i + 8, 8 * i:8 * i + 8],
                              in_=ct["Ww2s"][:])

        # ---- synthesize Sel / Ww1s / WpW1s on device -----------------
        # Sel[p, j] = 1 iff j // 16 == p  (i.e. 0 <= j - 16p <= 15)
        sel = cpool.tile([C, 16 * C], f16, tag="c_sel", name="c_sel")
        nc.gpsimd.memset(sel[:], 1.0)
        nc.gpsimd.affine_select(out=sel[:], in_=sel[:], compare_op=OP.is_ge,
                                fill=0.0, base=0, pattern=[[1, 16 * C]],
                                channel_multiplier=-16)
        nc.gpsimd.affine_select(out=sel[:], in_=sel[:], compare_op=OP.is_gt,
                                fill=0.0, base=16, pattern=[[-1, 16 * C]],
                                channel_multiplier=16)
        # E block i maps the 8 q-logit rows onto stacked partitions 8i+g
        etile = cpool.tile([G, 16 * C], f16, tag="c_e", name="c_e")
        nc.gpsimd.memset(etile[:], 0.0)
        nc.gpsimd.affine_select(out=etile[:], in_=etile[:],
                                compare_op=OP.not_equal, fill=1.0, base=0,
                                pattern=[[-8, 16], [1, C]],
                                channel_multiplier=-1)
        # Ww1s block i holds Ww1 at cols i*C + 8i .. +8 (rest zero)
        ww1s = cpool.tile([C, 16 * C], f16, tag="c_ww1s", name="c_ww1s")
        wpw1s = cpool.tile([C, 16 * C], f16, tag="c_wpw1s", name="c_wpw1s")
        nc.gpsimd.memset(ww1s[:], 0.0)
        nc.gpsimd.memset(wpw1s[:], 0.0)
        for i in range(16):
            c0 = i * C + 8 * i
            nc.vector.tensor_copy(out=ww1s[:, c0:c0 + 8], in_=ct["Ww1"][:])
            nc.vector.tensor_copy(out=wpw1s[:, c0:c0 + 8], in_=ct["P2W1"][:])

        outT = opool.tile([C, M_LOC], f32)

        for g in range(N_GROUPS):
            fT = gpool.tile([C, GROUP], f16, tag="fT")
            # gather + dequant + transpose this group's 8192 neighbor rows
            for blk in range(GROUP // C):
                gcol = g * (GROUP // C) + blk
                gt = gtpool.tile([C, ROWB], u8, tag="gt")
                nc.gpsimd.indirect_dma_start(
                    out=gt[:], out_offset=None,
                    in_=ctx2d,
                    in_offset=bass.IndirectOffsetOnAxis(
                        ap=knn32[:, gcol:gcol + 1], axis=0))
                sc32 = scpool.tile([C, 1], f32, tag="gsc")
                nc.vector.tensor_copy(out=sc32[:],
                                      in_=gt[:, 128:130].bitcast(f16))
                gt16 = g16pool.tile([C, C], f16, tag="g16")
                nc.scalar.activation(out=gt16[:], in_=gt[:, 0:128].bitcast(i8),
                                     func=AF.Identity, bias=0.0,
                                     scale=sc32[:])
                eng = nc.sync if blk % 2 == 0 else nc.scalar
                eng.dma_start_transpose(
                    out=fT[:, blk * C:(blk + 1) * C], in_=gt16[:])
            pT8 = p8pool.tile([3, GROUP // 2], f16, tag="pT8")
            nc.sync.dma_start(
                out=pT8[:],
                in_=view("posT8")[:, g * (GROUP // 2):(g + 1) * (GROUP // 2)])
            pT = gpool.tile([3, GROUP], f16, tag="pT")
            nc.vector.tensor_copy(out=pT[:], in_=pT8[:].bitcast(i8))
            valT = vpool.tile([C, GROUP], f32, tag="valp")
            stacked_ps = ps["stk"].tile([C, CHUNK], f32, tag="stk_t", name="stacked_ps")
            # -------- phase A: per chunk of 512 gathered rows ---------
            for i in range(16):
                ch = g * 16 + i              # global chunk id
                q0 = ch * 32                 # first query of chunk
                ctx = fT[:, i * CHUNK:(i + 1) * CHUNK]
                pos = pT[:, i * CHUNK:(i + 1) * CHUNK]
                # key = relu(bn(Wk.T @ ctx))
                k_ps = ps["kp"].tile([C, CHUNK], f32, tag="kp_t", name="k_ps")
                nc.tensor.matmul(out=k_ps[:], lhsT=ct["Wk"][:], rhs=ctx,
                                 start=True, stop=True)
                keyT = spool.tile([C, CHUNK], f16, tag="keyT")
                nc.scalar.activation(out=keyT[:], in_=k_ps[:], func=AF.Relu,
                                     bias=ct["bk"], scale=ct["sk"])
                # pebx = relu(bn(Wp1.T @ pos))
                pebx_ps = ps["px"].tile([C, CHUNK], f32, tag="px_t", name="pebx_ps")
                nc.tensor.matmul(out=pebx_ps[:], lhsT=ct["Wp1"][:], rhs=pos,
                                 start=True, stop=True)
                pebxT = spool.tile([C, CHUNK], f16, tag="pebxT")
                nc.scalar.activation(out=pebxT[:], in_=pebx_ps[:], func=AF.Relu,
                                     bias=ct["bp1"], scale=ct["sp1"])
                # val = Wv.T @ ctx + Wp2.T @ pebx (+ bv + bp2 via bias)
                v_ps = ps["vp"].tile([C, CHUNK], f32, tag="vp_t", name="v_ps")
                nc.tensor.matmul(out=v_ps[:], lhsT=ct["Wv"][:], rhs=ctx,
                                 start=True, stop=False)
                nc.tensor.matmul(out=v_ps[:], lhsT=ct["Wp2"][:], rhs=pebxT[:],
                                 start=False, stop=True)
                nc.scalar.activation(out=valT[:, i * CHUNK:(i + 1) * CHUNK],
                                     in_=v_ps[:], func=AF.Identity,
                                     bias=ct["bv"], scale=1.0)
                # w1 logits, stacked: Ww1.T @ (key - q + peb) with
                # peb folded via WpW1s = Wp2 @ Ww1s and -q via nqT
                q_rep = nq16[:, q0:q0 + 32].unsqueeze(2) \
                    .to_broadcast([G, 32, K])
                nc.tensor.matmul(out=stacked_ps[:],
                                 lhsT=ww1s[:, i * C:(i + 1) * C],
                                 rhs=keyT[:], start=(i == 0), stop=False,
                                 skip_group_check=True)
                nc.tensor.matmul(out=stacked_ps[:],
                                 lhsT=wpw1s[:, i * C:(i + 1) * C],
                                 rhs=pebxT[:], start=False, stop=False,
                                 skip_group_check=True)
                nc.tensor.matmul(out=stacked_ps[:],
                                 lhsT=etile[:, i * C:(i + 1) * C],
                                 rhs=q_rep, start=False, stop=(i == 15),
                                 skip_group_check=True)
            # -------- group tail: bn/relu, mm2, softmax ---------------
            stk_bn = spool.tile([C, CHUNK], f16, tag="stkbn")
            nc.scalar.activation(out=stk_bn[:], in_=stacked_ps[:], func=AF.Relu,
                                 bias=ct["bw1"], scale=ct["sw1"])
            w2_ps = ps["px"].tile([C, CHUNK], f32, tag="px_t", name="w2_ps")
            nc.tensor.matmul(out=w2_ps[:], lhsT=w2bd[:], rhs=stk_bn[:],
                             start=True, stop=True)
            mx = spool.tile([C, 32], f32, tag="mx")
            nc.vector.tensor_reduce(
                out=mx[:], in_=w2_ps[:].rearrange("p (m k) -> p m k", k=K),
                axis=mybir.AxisListType.X, op=OP.max)
            sm = spool.tile([C, CHUNK], f32, tag="sm")
            nc.vector.tensor_tensor(
                out=sm[:].rearrange("p (m k) -> p m k", k=K),
                in0=w2_ps[:].rearrange("p (m k) -> p m k", k=K),
                in1=mx[:].unsqueeze(2).to_broadcast([C, 32, K]),
                op=OP.subtract)
            e_t = spool.tile([C, CHUNK], f32, tag="e")
            nc.scalar.activation(out=e_t[:], in_=sm[:], func=AF.Exp)
            s_t = spool.tile([C, 32], f32, tag="s")
            nc.vector.tensor_reduce(
                out=s_t[:], in_=e_t[:].rearrange("p (m k) -> p m k", k=K),
                axis=mybir.AxisListType.X, op=OP.add)
            rinv = spool.tile([C, 32], f32, tag="rinv")
            nc.vector.reciprocal(out=rinv[:], in_=s_t[:])
            wf32 = spool.tile([C, CHUNK], f32, tag="wf32")
            nc.vector.tensor_tensor(
                out=wf32[:].rearrange("p (m k) -> p m k", k=K),
                in0=e_t[:].rearrange("p (m k) -> p m k", k=K),
                in1=rinv[:].unsqueeze(2).to_broadcast([C, 32, K]),
                op=OP.mult)
            wfin = spool.tile([C, CHUNK], f16, tag="wfin")
            nc.scalar.activation(out=wfin[:], in_=wf32[:], func=AF.Identity)
            # -------- phase B: weighted sum per chunk -----------------
            for i in range(16):
                ch = g * 16 + i
                wrep_ps = ps["wr"].tile([C, CHUNK], f32, tag="wr_t", name="wrep_ps")
                nc.tensor.matmul(out=wrep_ps[:],
                                 lhsT=sel[:, i * C:(i + 1) * C],
                                 rhs=wfin[:], start=True, stop=True)
                prod = spool.tile([C, CHUNK], f32, tag="prod")
                nc.vector.tensor_tensor(out=prod[:],
                                        in0=valT[:, i * CHUNK:(i + 1) * CHUNK],
                                        in1=wrep_ps[:], op=OP.mult)
                nc.vector.tensor_reduce(
                    out=outT[:, ch * 32:(ch + 1) * 32],
                    in_=prod[:].rearrange("p (m k) -> p m k", k=K),
                    axis=mybir.AxisListType.X, op=OP.add)

        # -------- output: per-channel u8 quantization -----------------
        m_t = opool.tile([C, 1], f32, tag="omax", name="omax")
        mn_t = opool.tile([C, 1], f32, tag="omin", name="omin")
        nc.vector.tensor_reduce(out=m_t[:], in_=outT[:],
                                axis=mybir.AxisListType.X, op=OP.max)
        nc.vector.tensor_reduce(out=mn_t[:], in_=outT[:],
                                axis=mybir.AxisListType.X, op=OP.min)
        nc.vector.tensor_scalar_mul(out=mn_t[:], in0=mn_t[:], scalar1=-1.0)
        nc.vector.tensor_tensor(out=m_t[:], in0=m_t[:], in1=mn_t[:],
                                op=OP.max)
        nc.vector.tensor_scalar_max(out=m_t[:], in0=m_t[:], scalar1=1e-20)
        rs = opool.tile([C, 1], f32, tag="orsc", name="orsc")
        nc.vector.reciprocal(out=rs[:], in_=m_t[:])
        # 126.99 (not 127) so the row-max element lands strictly below
        # 255.5 after the +128.5 shift even if reciprocal rounds up --
        # keeps the u8 convert away from any wrap/saturate edge.
        nc.vector.tensor_scalar_mul(out=rs[:], in0=rs[:], scalar1=126.99)
        y8 = opool.tile([C, M_LOC], u8, tag="oy8", name="oy8")
        nc.vector.tensor_scalar(out=y8[:], in0=outT[:], scalar1=rs[:],
                                scalar2=128.5, op0=OP.mult, op1=OP.add)
        nc.sync.dma_start(out=out_d[:, 0:M_LOC], in_=y8[:])
        nc.sync.dma_start(out=out_d[:, M_LOC:M_LOC + 4].bitcast(f32),
                          in_=m_t[:])

    nc.compile()
    return nc


def _make_runner(nc):
    """PJRT dispatch for the compiled Bass module, mirroring
    bass2jax.run_bass_via_pjrt but keeping the (ignored) donated-zero
    output operands device-resident so each call uploads only the blob."""
    import jax
    from jax.sharding import Mesh, PartitionSpec, NamedSharding
    from jax.experimental.shard_map import shard_map
    from concourse import bass2jax, mybir

    bass2jax.install_neuronx_cc_hook()
    assert nc.dbg_addr is None or not nc.dbg_callbacks

    partition_name = (nc.partition_id_tensor.name
                      if nc.partition_id_tensor else None)
    in_names, in_shapes = [], {}
    out_names, out_avals, zero_outs = [], [], []
    for alloc in nc.m.functions[0].allocations:
        if not isinstance(alloc, mybir.MemoryLocationSet):
            continue
        name = alloc.memorylocations[0].name
        if alloc.kind == "ExternalInput":
            if name != partition_name:
                in_names.append(name)
                in_shapes[name] = (tuple(alloc.tensor_shape),
                                   mybir.dt.np(alloc.dtype))
        elif alloc.kind == "ExternalOutput":
            shape = tuple(alloc.tensor_shape)
            dtype = mybir.dt.np(alloc.dtype)
            out_names.append(name)
            out_avals.append(jax.core.ShapedArray(shape, dtype))
            zero_outs.append(np.zeros((N_CORES * shape[0], *shape[1:]), dtype))
    n_params = len(in_names)
    all_names = tuple(in_names + out_names)

    def _body(*args):
        operands = list(args)
        if partition_name is not None:
            operands.append(bass2jax.partition_id_tensor())
        outs = bass2jax._bass_exec_p.bind(
            *operands,
            out_avals=tuple(out_avals),
            in_names=all_names + ((partition_name,) if partition_name else ()),
            out_names=tuple(out_names),
            lowering_input_output_aliases=(),
            sim_require_finite=True,
            sim_require_nnan=True,
            nc=nc,
        )
        return tuple(outs)

    devices = jax.devices()[:N_CORES]
    assert len(devices) == N_CORES
    mesh = Mesh(np.asarray(devices), ("core",))
    nspec = NamedSharding(mesh, PartitionSpec("core"))
    in_specs = (PartitionSpec("core"),) * (n_params + len(out_names))
    out_specs = (PartitionSpec("core"),) * len(out_names)
    fn = jax.jit(
        shard_map(_body, mesh=mesh, in_specs=in_specs, out_specs=out_specs,
                  check_rep=False),
        keep_unused=True,
    )
    zeros_dev = [jax.device_put(z, nspec) for z in zero_outs]

    def run(in_maps):
        concat = [
            np.concatenate([np.asarray(in_maps[c][nm]) for c in range(N_CORES)],
                           axis=0)
            for nm in in_names
        ]
        outs = fn(*concat, *zeros_dev)
        return [
            {nm: np.asarray(outs[i]).reshape(N_CORES, *out_avals[i].shape)[c]
             for i, nm in enumerate(out_names)}
            for c in range(N_CORES)
        ]

    return run


def _prep_inputs(inputs):
    """Host-side marshaling: quantize+shard context, gather positions,
    fp16 blob pack."""
    f = np.float32
    h = np.float16
    ctx_f = np.asarray(inputs["context_feat"], f)
    ctx_c = np.asarray(inputs["context_coord"], f)

    knn = np.asarray(inputs["knn_indexes"])
    knn = np.where(knn < 0, 0, knn).astype(np.int32)
    # compact the table to referenced rows only, renumbering knn
    used = np.unique(knn)                                      # sorted
    cap = (len(used) + 128 + 1023) // 1024 * 1024
    _set_layout(cap)
    cap = _CAP
    knn_r = np.searchsorted(used, knn).astype(np.int32)        # [M, K]

    # int8 per-row quantized context records: 128 x i8 | f16 row scale
    rowmax = np.abs(ctx_f).max(axis=1)
    rsc = np.maximum(rowmax, 1e-12) / 127.0                    # [N] f32
    q8 = np.clip(np.round(ctx_f / rsc[:, None]), -127, 127).astype(np.int8)
    tab = np.zeros((cap, ROWB), np.uint8)
    tab[:len(used), :C] = q8[used].view(np.uint8)
    tab[:len(used), C:C + 2] = rsc[used].astype(h).reshape(-1, 1) \
        .view(np.uint8)

    s = lambda g_: (np.asarray(g_, f) / np.sqrt(np.float32(1.0 + EPS_BN)))
    Wq = np.asarray(inputs["Wq"], f); Wk = np.asarray(inputs["Wk"], f)
    Wv = np.asarray(inputs["Wv"], f)
    Wp1 = np.asarray(inputs["Wp1"], f); Wp2 = np.asarray(inputs["Wp2"], f)
    Ww1 = np.asarray(inputs["Ww1"], f); Ww2 = np.asarray(inputs["Ww2"], f)

    sq = s(inputs["gq"]); bq = sq * inputs["bq"] + np.asarray(inputs["betaq"], f)
    sk = s(inputs["gk"]); bk = sk * inputs["bk"] + np.asarray(inputs["betak"], f)
    sp1 = s(inputs["gp1"])
    bp1 = sp1 * inputs["bp1"] + np.asarray(inputs["betap1"], f)
    bv = np.asarray(inputs["bv"], f) + np.asarray(inputs["bp2"], f)  # val bias
    # stacked bn for w1: row 8i+g ; fold bp2@Ww1 into bias
    sw1_g = s(inputs["gw1"])                                   # [G]
    bw1_g = (sw1_g * (np.asarray(inputs["bw1"], f)
                      + np.asarray(inputs["bp2"], f) @ Ww1)
             + np.asarray(inputs["betaw1"], f))                # [G]
    sw1 = np.tile(sw1_g, 16).astype(f)
    bw1 = np.tile(bw1_g, 16).astype(f)

    P2W1 = (Wp2 @ Ww1).astype(f)                               # [C, G]

    qf = np.asarray(inputs["query_feat"], f)
    qc = np.asarray(inputs["query_coord"], f)

    q_full = np.maximum(sq * (qf @ Wq) + bq, 0.0)          # [M, C]
    nqwT = (-(q_full @ Ww1)).T.astype(f)                   # [G, M]
    snq = np.float32(max(np.abs(nqwT).max(), 1e-12) / 127.0)
    nq8 = np.round(nqwT / snq).astype(np.int8)             # [G, M]
    fixed = {"Wk": Wk, "Wv": Wv, "Wp2": Wp2, "Ww2s": Ww2,
             "Ww1": Ww1, "P2W1": P2W1, "Wp1": Wp1}
    fixed16 = {nm: np.asarray(v, f).astype(h).ravel() for nm, v in fixed.items()}

    tab_loc = cap // N_CORES
    in_maps = []
    for c in range(N_CORES):
        sl = slice(c * M_LOC, (c + 1) * M_LOC)
        idx = knn[sl].reshape(-1)          # original ids, for coord gather
        knn_t = knn_r[sl].reshape(R_LOC // C, C).T.copy()  # renumbered, i32
        pos = (ctx_c[idx] - np.repeat(qc[sl], K, axis=0))        # [R_LOC, 3]
        psc = np.float32(max(np.abs(pos).max(), 1e-12) / 127.0)
        p8 = np.round(pos / psc).astype(np.int8).T               # [3, R_LOC]
        # fold the pos dequant scale into the positional-BN scale
        scal = np.stack([sq, bq, sk, bk, bv, sp1 * psc, bp1, sw1, bw1,
                         np.full(C, snq, f)], axis=1)            # [C, 10]
        blob = np.empty(_NTOT, h)
        pieces = dict(fixed16)
        pieces["ctxtab"] = np.ascontiguousarray(
            tab[c * tab_loc:(c + 1) * tab_loc]).reshape(-1).view(h)
        pieces["nqwT8"] = np.ascontiguousarray(nq8[:, sl]).reshape(-1).view(h)
        pieces["knn16"] = knn_t.view(h).ravel()
        pieces["posT8"] = np.ascontiguousarray(p8).reshape(-1).view(h)
        pieces["scal"] = scal.astype(h).ravel()
        for nm, (p_, c_) in _LAYOUT:
            off = _OFFS[nm]
            blob[off:off + p_ * c_] = pieces[nm]
        in_maps.append({"blob": blob})
    return in_maps


def _get():
    global _compiled
    if _compiled is None:
        nc = _build()
        _compiled = (nc, _make_runner(nc))
    return _compiled


def _decode(res):
    """u8 per-channel quantized device output -> full [M, C] fp32."""
    outs = []
    for c in range(N_CORES):
        a = res[c]["out"]                                  # [C, M_LOC+4] u8
        m_ = np.ascontiguousarray(a[:, M_LOC:M_LOC + 4]).view(np.float32)
        y = a[:, :M_LOC].astype(np.float32)
        outs.append(((y - 128.0) * (m_ / 126.99)).T)
    return np.ascontiguousarray(np.concatenate(outs, axis=0).astype(np.float32))


def kernel(**inputs):
    in_maps = _prep_inputs(inputs)   # sets the table cap before first build
    nc, run = _get()
    return _decode(run(in_maps))


# revision 33
# speedup vs baseline: 1.1871x; 1.0359x over previous
"""KNN grouped-vector-attention pool kernel for 8 Trainium2 NeuronCores.

Strategy: shard queries M=16384 across 8 cores (2048 each). The context
feature table is sharded across cores and reassembled on device with an HBM
AllGather; each core then resolves its own KNN gathers locally via indirect
DMA and XBAR DMA-transposes into channel-major layout.

Interconnect to the device is the bottleneck (slow tunneled PJRT link), so
the wire format is aggressively compressed:
  * only context rows actually referenced by knn_indexes ship (~86.5% of N
    for random indexes; knn renumbered on host), as 130-byte records:
    128 x int8 (per-row absmax quantized) + the fp16 row scale. Dequant
    happens on device right after each 128-row gather (ACT engine,
    per-partition scale).
    The AllGather moves the records typed as u16: the collective datapath
    routes elements through f32, so u8/u16 round-trip exactly while f16
    wires flush denormals and i32 wires lose low mantissa bits.
  * relative positions ship as int8 with a per-core global scale folded
    into the positional-BN scale scalar.
  * the output returns as uint8 (per-channel absmax quantized, bias +128)
    with the per-channel fp32 scale appended to the row.
  * the query projection is rank-reduced on the host: q only enters the
    logits as q@Ww1 [M,8], so an [8,M] fp16 stripe uploads instead of the
    [128,M] query block.
  * Ww2 uploads as a single 8x8 tile; its 16-block block-diagonal expansion
    is assembled on device. Sel / E / Ww1s / WpW1s selector matrices are
    synthesized on device from tiny seeds.
All per-core inputs pack into one contiguous fp16 blob (~2.4MB) so the
host->device path pays a single transfer per core. The dispatch path is a
custom PJRT runner that keeps the donated-zero output buffers device-
resident, avoiding run_bass_kernel_spmd's per-call zero upload. All matmuls
run fp16 x fp16 with fp32 PSUM accumulation.
"""
import sys
sys.path.insert(0, '/opt/trn_rl_repo')
import numpy as np

N_CORES = 8
M, N, K, C, G = 16384, 131072, 16, 128, 8
M_LOC = M // N_CORES          # 2048 queries per core
R_LOC = M_LOC * K             # 32768 gathered rows per core
N_LOC = N // N_CORES          # 16384 context rows uploaded per core
CHUNK = 512                   # rows per compute chunk (one PSUM bank)
GROUP = 16 * CHUNK            # 8192 rows per stacked group
N_GROUPS = R_LOC // GROUP     # 4
ROWB = 130                    # context record: 128 int8 + fp16 row scale
EPS_BN = 1e-5

# The table holds only the context rows actually referenced by knn_indexes
# (~86.5% of N for random indexes); knn is renumbered on the host. The cap
# is fixed at first call (shapes are baked into the compiled kernel).
_CAP = None
_LAYOUT = None
_OFFS = None
_NTOT = None


def _set_layout(cap):
    global _CAP, _LAYOUT, _OFFS, _NTOT
    if _CAP is not None:
        assert cap <= _CAP, f"table cap {cap} exceeds compiled {_CAP}"
        return
    _CAP = cap
    # per-core table contribution: cap/8 context records + 128 pad rows
    # whose first 12288 bytes carry this core's 1/8 column-slice of the
    # packed Wk|Wv|Wp2 block (deduplicated via the AllGather)
    rows2 = cap // N_CORES + 128
    # blob layout: (name, (partitions, cols)) packed row-major, fp16 units
    _LAYOUT = [
        ("ctxtab", (C, rows2 * ROWB // 2 // C)),  # u8 records + param rows
        ("nqwT8", (G, M_LOC // 2)),  # -(relu(bn(qf@Wq)) @ Ww1).T int8
        ("Ww2s", (G, G)),
        ("Ww1", (C, G)), ("P2W1", (C, G)),
        ("scal", (C, 10)),           # sq,bq,sk,bk,bv,sp1,bp1,sw1,bw1,snq
        ("knn16", (C, R_LOC // C * 2)),  # [128,256] i32 KNN blocks, fp16 bits
        ("posT8", (3, R_LOC // 2)),  # [3, R_LOC] int8, fp16 bits
        ("Wp1", (3, C)),
    ]
    _OFFS = {}
    tot = 0
    for nm, (p, c) in _LAYOUT:
        _OFFS[nm] = tot
        tot += p * c
    _NTOT = tot


_compiled = None


def _build():
    from concourse import bacc, bass, mybir
    import concourse.tile as tile

    f32 = mybir.dt.float32
    f16 = mybir.dt.float16
    i32 = mybir.dt.int32
    i8 = mybir.dt.int8
    u8 = mybir.dt.uint8
    u16 = mybir.dt.uint16
    AF = mybir.ActivationFunctionType
    OP = mybir.AluOpType

    nc = bacc.Bacc("TRN2", target_bir_lowering=False, debug=False,
                   num_devices=N_CORES)

    blob = nc.dram_tensor("blob", (_NTOT,), f16, kind="ExternalInput").ap()
    out_d = nc.dram_tensor("out", (C, M_LOC + 4), u8,
                           kind="ExternalOutput").ap()

    def view(nm):
        p, c = dict(_LAYOUT)[nm]
        off = _OFFS[nm]
        return blob[off:off + p * c].rearrange("(p c) -> p c", p=p)

    from contextlib import ExitStack
    est = ExitStack()
    with tile.TileContext(nc) as tc, est:
        dpool = est.enter_context(tc.tile_pool(name="dram", bufs=1, space="DRAM"))
        cpool = est.enter_context(tc.tile_pool(name="const", bufs=1))
        gtpool = est.enter_context(tc.tile_pool(name="gt", bufs=8))
        g16pool = est.enter_context(tc.tile_pool(name="g16", bufs=8))
        scpool = est.enter_context(tc.tile_pool(name="gsc", bufs=8))
        gpool = est.enter_context(tc.tile_pool(name="gath", bufs=2))
        p8pool = est.enter_context(tc.tile_pool(name="p8st", bufs=2))
        vpool = est.enter_context(tc.tile_pool(name="valp", bufs=2))
        spool = est.enter_context(tc.tile_pool(name="work", bufs=2))
        opool = est.enter_context(tc.tile_pool(name="outp", bufs=1))
        ps = {}
        for nm, nb in [("kp", 2), ("px", 2), ("vp", 2), ("stk", 1), ("wr", 1)]:
            ps[nm] = est.enter_context(tc.tile_pool(name=nm, bufs=nb, space="PSUM"))

        # ---- AllGather the context record table in HBM ----------------
        # wire dtype MUST be an int type <= 16 bits: the collective datapath
        # routes elements through f32 (u16/u8 round-trip exactly; f16
        # denormals get flushed and i32 loses low mantissa bits)
        tp, tc_ = dict(_LAYOUT)["ctxtab"]
        ib = dpool.tile([tp, tc_], u16, tag="ib", name="ib")
        ob = dpool.tile([tp, tc_ * N_CORES], u16, tag="ob", name="ob",
                        addr_space="Shared")
        nc.gpsimd.dma_start(ib[:], view("ctxtab").bitcast(u16))
        nc.gpsimd.collective_compute(
            "AllGather", OP.bypass,
            replica_groups=[list(range(N_CORES))],
            ins=[ib.opt()], outs=[ob.opt()])
        # reinterpret the gathered flat buffer as [N, 132] u8 records
        ctx2d = ob[:].bitcast(u8).rearrange("p (r c) -> (p r) c", c=ROWB)

        # ---- constants into SBUF -------------------------------------
        ct = {}
        for nm in ("nqwT8", "Ww2s", "Ww1", "P2W1",
                   "scal", "knn16", "Wp1"):
            p, c = dict(_LAYOUT)[nm]
            ct[nm] = cpool.tile([p, c], f16, tag=f"c_{nm}", name=f"c_{nm}")
            nc.sync.dma_start(out=ct[nm][:], in_=view(nm))
        # reassemble the deduplicated Wk|Wv|Wp2 pack: each core shipped a
        # [128, 96B] column slice in the first 12288 bytes of its 128
        # param pad rows at the tail of its table contribution
        rows_loc = _CAP // N_CORES
        stride_b = (rows_loc + 128) * ROWB
        obflat = ob[:].bitcast(u8).rearrange("p x -> (p x)")
        wtile = cpool.tile([C, 768], u8, tag="c_wpack", name="c_wpack")
        for cc in range(N_CORES):
            off = cc * stride_b + rows_loc * ROWB
            nc.sync.dma_start(
                out=wtile[:, cc * 96:(cc + 1) * 96],
                in_=obflat[off:off + 12288].rearrange("(p b) -> p b", p=C))
        wf16 = wtile[:].bitcast(f16)             # [C, 384] = Wk|Wv|Wp2
        wK, wV, wP2 = wf16[:, 0:C], wf16[:, C:2 * C], wf16[:, 2 * C:3 * C]
        knn32 = ct["knn16"][:].bitcast(i32)          # [128, R_LOC/128] i32
        # fp16 scalars -> f32 working copy; per-scalar column APs
        scal32 = cpool.tile([C, 10], f32, tag="c_scal32", name="c_scal32")
        nc.vector.tensor_copy(out=scal32[:], in_=ct["scal"][:])
        for j, nm in enumerate(("sq", "bq", "sk", "bk", "bv", "sp1", "bp1",
                                "sw1", "bw1")):
            ct[nm] = scal32[:, j:j + 1]
        # dequantize the int8 q stripe: nq16 = nq8 * snq
        nq16 = cpool.tile([G, M_LOC], f16, tag="c_nq16", name="c_nq16")
        nc.vector.tensor_copy(out=nq16[:], in_=ct["nqwT8"][:].bitcast(i8))
        nc.vector.tensor_scalar(out=nq16[:], in0=nq16[:],
                                scalar1=scal32[0:G, 9:10], scalar2=None,
                                op0=OP.mult)

        # W2bd: 16-block block-diagonal expansion of Ww2 (8x8)
        w2bd = cpool.tile([C, C], f16, tag="c_w2bd", name="c_w2bd")
        nc.gpsimd.memset(w2bd[:], 0.0)
        for i in range(16):
            nc.sync.dma_start(out=w2bd[8 * i:8 * i + 8, 8 * i:8 * i + 8],
                              in_=ct["Ww2s"][:])

        # ---- synthesize Sel / Ww1s / WpW1s on device -----------------
        # Sel[p, j] = 1 iff j // 16 == p  (i.e. 0 <= j - 16p <= 15)
        sel = cpool.tile([C, 16 * C], f16, tag="c_sel", name="c_sel")
        nc.gpsimd.memset(sel[:], 1.0)
        nc.gpsimd.affine_select(out=sel[:], in_=sel[:], compare_op=OP.is_ge,
                                fill=0.0, base=0, pattern=[[1, 16 * C]],
                                channel_multiplier=-16)
        nc.gpsimd.affine_select(out=sel[:], in_=sel[:], compare_op=OP.is_gt,
                                fill=0.0, base=16, pattern=[[-1, 16 * C]],
                                channel_multiplier=16)
        # E block i maps the 8 q-logit rows onto stacked partitions 8i+g
        etile = cpool.tile([G, 16 * C], f16, tag="c_e", name="c_e")
        nc.gpsimd.memset(etile[:], 0.0)
        nc.gpsimd.affine_select(out=etile[:], in_=etile[:],
                                compare_op=OP.not_equal, fill=1.0, base=0,
                                pattern=[[-8, 16], [1, C]],
                                channel_multiplier=-1)
        # Ww1s block i holds Ww1 at cols i*C + 8i .. +8 (rest zero)
        ww1s = cpool.tile([C, 16 * C], f16, tag="c_ww1s", name="c_ww1s")
        wpw1s = cpool.tile([C, 16 * C], f16, tag="c_wpw1s", name="c_wpw1s")
        nc.gpsimd.memset(ww1s[:], 0.0)
        nc.gpsimd.memset(wpw1s[:], 0.0)
        for i in range(16):
            c0 = i * C + 8 * i
            nc.vector.tensor_copy(out=ww1s[:, c0:c0 + 8], in_=ct["Ww1"][:])
            nc.vector.tensor_copy(out=wpw1s[:, c0:c0 + 8], in_=ct["P2W1"][:])

        outT = opool.tile([C, M_LOC], f32)

        for g in range(N_GROUPS):
            fT = gpool.tile([C, GROUP], f16, tag="fT")
            # gather + dequant + transpose this group's 8192 neighbor rows
            for blk in range(GROUP // C):
                gcol = g * (GROUP // C) + blk
                gt = gtpool.tile([C, ROWB], u8, tag="gt")
                nc.gpsimd.indirect_dma_start(
                    out=gt[:], out_offset=None,
                    in_=ctx2d,
                    in_offset=bass.IndirectOffsetOnAxis(
                        ap=knn32[:, gcol:gcol + 1], axis=0))
                sc32 = scpool.tile([C, 1], f32, tag="gsc")
                nc.vector.tensor_copy(out=sc32[:],
                                      in_=gt[:, 128:130].bitcast(f16))
                gt16 = g16pool.tile([C, C], f16, tag="g16")
                nc.scalar.activation(out=gt16[:], in_=gt[:, 0:128].bitcast(i8),
                                     func=AF.Identity, bias=0.0,
                                     scale=sc32[:])
                eng = nc.sync if blk % 2 == 0 else nc.scalar
                eng.dma_start_transpose(
                    out=fT[:, blk * C:(blk + 1) * C], in_=gt16[:])
            pT8 = p8pool.tile([3, GROUP // 2], f16, tag="pT8")
            nc.sync.dma_start(
                out=pT8[:],
                in_=view("posT8")[:, g * (GROUP // 2):(g + 1) * (GROUP // 2)])
            pT = gpool.tile([3, GROUP], f16, tag="pT")
            nc.vector.tensor_copy(out=pT[:], in_=pT8[:].bitcast(i8))
            valT = vpool.tile([C, GROUP], f32, tag="valp")
            stacked_ps = ps["stk"].tile([C, CHUNK], f32, tag="stk_t", name="stacked_ps")
            # -------- phase A: per chunk of 512 gathered rows ---------
            for i in range(16):
                ch = g * 16 + i              # global chunk id
                q0 = ch * 32                 # first query of chunk
                ctx = fT[:, i * CHUNK:(i + 1) * CHUNK]
                pos = pT[:, i * CHUNK:(i + 1) * CHUNK]
                # key = relu(bn(Wk.T @ ctx))
                k_ps = ps["kp"].tile([C, CHUNK], f32, tag="kp_t", name="k_ps")
                nc.tensor.matmul(out=k_ps[:], lhsT=wK, rhs=ctx,
                                 start=True, stop=True)
                keyT = spool.tile([C, CHUNK], f16, tag="keyT")
                nc.scalar.activation(out=keyT[:], in_=k_ps[:], func=AF.Relu,
                                     bias=ct["bk"], scale=ct["sk"])
                # pebx = relu(bn(Wp1.T @ pos))
                pebx_ps = ps["px"].tile([C, CHUNK], f32, tag="px_t", name="pebx_ps")
                nc.tensor.matmul(out=pebx_ps[:], lhsT=ct["Wp1"][:], rhs=pos,
                                 start=True, stop=True)
                pebxT = spool.tile([C, CHUNK], f16, tag="pebxT")
                nc.scalar.activation(out=pebxT[:], in_=pebx_ps[:], func=AF.Relu,
                                     bias=ct["bp1"], scale=ct["sp1"])
                # val = Wv.T @ ctx + Wp2.T @ pebx (+ bv + bp2 via bias)
                v_ps = ps["vp"].tile([C, CHUNK], f32, tag="vp_t", name="v_ps")
                nc.tensor.matmul(out=v_ps[:], lhsT=wV, rhs=ctx,
                                 start=True, stop=False)
                nc.tensor.matmul(out=v_ps[:], lhsT=wP2, rhs=pebxT[:],
                                 start=False, stop=True)
                nc.scalar.activation(out=valT[:, i * CHUNK:(i + 1) * CHUNK],
                                     in_=v_ps[:], func=AF.Identity,
                                     bias=ct["bv"], scale=1.0)
                # w1 logits, stacked: Ww1.T @ (key - q + peb) with
                # peb folded via WpW1s = Wp2 @ Ww1s and -q via nqT
                q_rep = nq16[:, q0:q0 + 32].unsqueeze(2) \
                    .to_broadcast([G, 32, K])
                nc.tensor.matmul(out=stacked_ps[:],
                                 lhsT=ww1s[:, i * C:(i + 1) * C],
                                 rhs=keyT[:], start=(i == 0), stop=False,
                                 skip_group_check=True)
                nc.tensor.matmul(out=stacked_ps[:],
                                 lhsT=wpw1s[:, i * C:(i + 1) * C],
                                 rhs=pebxT[:], start=False, stop=False,
                                 skip_group_check=True)
                nc.tensor.matmul(out=stacked_ps[:],
                                 lhsT=etile[:, i * C:(i + 1) * C],
                                 rhs=q_rep, start=False, stop=(i == 15),
                                 skip_group_check=True)
            # -------- group tail: bn/relu, mm2, softmax ---------------
            stk_bn = spool.tile([C, CHUNK], f16, tag="stkbn")
            nc.scalar.activation(out=stk_bn[:], in_=stacked_ps[:], func=AF.Relu,
                                 bias=ct["bw1"], scale=ct["sw1"])
            w2_ps = ps["px"].tile([C, CHUNK], f32, tag="px_t", name="w2_ps")
            nc.tensor.matmul(out=w2_ps[:], lhsT=w2bd[:], rhs=stk_bn[:],
                             start=True, stop=True)
            mx = spool.tile([C, 32], f32, tag="mx")
            nc.vector.tensor_reduce(
                out=mx[:], in_=w2_ps[:].rearrange("p (m k) -> p m k", k=K),
                axis=mybir.AxisListType.X, op=OP.max)
            sm = spool.tile([C, CHUNK], f32, tag="sm")
            nc.vector.tensor_tensor(
                out=sm[:].rearrange("p (m k) -> p m k", k=K),
                in0=w2_ps[:].rearrange("p (m k) -> p m k", k=K),
                in1=mx[:].unsqueeze(2).to_broadcast([C, 32, K]),
                op=OP.subtract)
            e_t = spool.tile([C, CHUNK], f32, tag="e")
            nc.scalar.activation(out=e_t[:], in_=sm[:], func=AF.Exp)
            s_t = spool.tile([C, 32], f32, tag="s")
            nc.vector.tensor_reduce(
                out=s_t[:], in_=e_t[:].rearrange("p (m k) -> p m k", k=K),
                axis=mybir.AxisListType.X, op=OP.add)
            rinv = spool.tile([C, 32], f32, tag="rinv")
            nc.vector.reciprocal(out=rinv[:], in_=s_t[:])
            wf32 = spool.tile([C, CHUNK], f32, tag="wf32")
            nc.vector.tensor_tensor(
                out=wf32[:].rearrange("p (m k) -> p m k", k=K),
                in0=e_t[:].rearrange("p (m k) -> p m k", k=K),
                in1=rinv[:].unsqueeze(2).to_broadcast([C, 32, K]),
                op=OP.mult)
            wfin = spool.tile([C, CHUNK], f16, tag="wfin")
            nc.scalar.activation(out=wfin[:], in_=wf32[:], func=AF.Identity)
            # -------- phase B: weighted sum per chunk -----------------
            for i in range(16):
                ch = g * 16 + i
                wrep_ps = ps["wr"].tile([C, CHUNK], f32, tag="wr_t", name="wrep_ps")
                nc.tensor.matmul(out=wrep_ps[:],
                                 lhsT=sel[:, i * C:(i + 1) * C],
                                 rhs=wfin[:], start=True, stop=True)
                prod = spool.tile([C, CHUNK], f32, tag="prod")
                nc.vector.tensor_tensor(out=prod[:],
                                        in0=valT[:, i * CHUNK:(i + 1) * CHUNK],
                                        in1=wrep_ps[:], op=OP.mult)
                nc.vector.tensor_reduce(
                    out=outT[:, ch * 32:(ch + 1) * 32],
                    in_=prod[:].rearrange("p (m k) -> p m k", k=K),
                    axis=mybir.AxisListType.X, op=OP.add)

        # -------- output: per-channel u8 quantization -----------------
        m_t = opool.tile([C, 1], f32, tag="omax", name="omax")
        mn_t = opool.tile([C, 1], f32, tag="omin", name="omin")
        nc.vector.tensor_reduce(out=m_t[:], in_=outT[:],
                                axis=mybir.AxisListType.X, op=OP.max)
        nc.vector.tensor_reduce(out=mn_t[:], in_=outT[:],
                                axis=mybir.AxisListType.X, op=OP.min)
        nc.vector.tensor_scalar_mul(out=mn_t[:], in0=mn_t[:], scalar1=-1.0)
        nc.vector.tensor_tensor(out=m_t[:], in0=m_t[:], in1=mn_t[:],
                                op=OP.max)
        nc.vector.tensor_scalar_max(out=m_t[:], in0=m_t[:], scalar1=1e-20)
        rs = opool.tile([C, 1], f32, tag="orsc", name="orsc")
        nc.vector.reciprocal(out=rs[:], in_=m_t[:])
        # 126.99 (not 127) so the row-max element lands strictly below
        # 255.5 after the +128.5 shift even if reciprocal rounds up --
        # keeps the u8 convert away from any wrap/saturate edge.
        nc.vector.tensor_scalar_mul(out=rs[:], in0=rs[:], scalar1=126.99)
        y8 = opool.tile([C, M_LOC], u8, tag="oy8", name="oy8")
        nc.vector.tensor_scalar(out=y8[:], in0=outT[:], scalar1=rs[:],
                                scalar2=128.5, op0=OP.mult, op1=OP.add)
        nc.sync.dma_start(out=out_d[:, 0:M_LOC], in_=y8[:])
        nc.sync.dma_start(out=out_d[:, M_LOC:M_LOC + 4].bitcast(f32),
                          in_=m_t[:])

    nc.compile()
    return nc


def _make_runner(nc):
    """PJRT dispatch for the compiled Bass module, mirroring
    bass2jax.run_bass_via_pjrt but keeping the (ignored) donated-zero
    output operands device-resident so each call uploads only the blob."""
    import jax
    from jax.sharding import Mesh, PartitionSpec, NamedSharding
    from jax.experimental.shard_map import shard_map
    from concourse import bass2jax, mybir

    bass2jax.install_neuronx_cc_hook()
    assert nc.dbg_addr is None or not nc.dbg_callbacks

    partition_name = (nc.partition_id_tensor.name
                      if nc.partition_id_tensor else None)
    in_names, in_shapes = [], {}
    out_names, out_avals, zero_outs = [], [], []
    for alloc in nc.m.functions[0].allocations:
        if not isinstance(alloc, mybir.MemoryLocationSet):
            continue
        name = alloc.memorylocations[0].name
        if alloc.kind == "ExternalInput":
            if name != partition_name:
                in_names.append(name)
                in_shapes[name] = (tuple(alloc.tensor_shape),
                                   mybir.dt.np(alloc.dtype))
        elif alloc.kind == "ExternalOutput":
            shape = tuple(alloc.tensor_shape)
            dtype = mybir.dt.np(alloc.dtype)
            out_names.append(name)
            out_avals.append(jax.core.ShapedArray(shape, dtype))
            zero_outs.append(np.zeros((N_CORES * shape[0], *shape[1:]), dtype))
    n_params = len(in_names)
    all_names = tuple(in_names + out_names)

    def _body(*args):
        operands = list(args)
        if partition_name is not None:
            operands.append(bass2jax.partition_id_tensor())
        outs = bass2jax._bass_exec_p.bind(
            *operands,
            out_avals=tuple(out_avals),
            in_names=all_names + ((partition_name,) if partition_name else ()),
            out_names=tuple(out_names),
            lowering_input_output_aliases=(),
            sim_require_finite=True,
            sim_require_nnan=True,
            nc=nc,
        )
        return tuple(outs)

    devices = jax.devices()[:N_CORES]
    assert len(devices) == N_CORES
    mesh = Mesh(np.asarray(devices), ("core",))
    nspec = NamedSharding(mesh, PartitionSpec("core"))
    in_specs = (PartitionSpec("core"),) * (n_params + len(out_names))
    out_specs = (PartitionSpec("core"),) * len(out_names)
    fn = jax.jit(
        shard_map(_body, mesh=mesh, in_specs=in_specs, out_specs=out_specs,
                  check_rep=False),
        keep_unused=True,
    )
    zeros_dev = [jax.device_put(z, nspec) for z in zero_outs]

    def run(in_maps):
        concat = [
            np.concatenate([np.asarray(in_maps[c][nm]) for c in range(N_CORES)],
                           axis=0)
            for nm in in_names
        ]
        outs = fn(*concat, *zeros_dev)
        return [
            {nm: np.asarray(outs[i]).reshape(N_CORES, *out_avals[i].shape)[c]
             for i, nm in enumerate(out_names)}
            for c in range(N_CORES)
        ]

    return run


def _prep_inputs(inputs):
    """Host-side marshaling: quantize+shard context, gather positions,
    fp16 blob pack."""
    f = np.float32
    h = np.float16
    ctx_f = np.asarray(inputs["context_feat"], f)
    ctx_c = np.asarray(inputs["context_coord"], f)

    knn = np.asarray(inputs["knn_indexes"])
    knn = np.where(knn < 0, 0, knn).astype(np.int32)
    # compact the table to referenced rows only, renumbering knn
    used = np.unique(knn)                                      # sorted
    cap = (len(used) + 128 + 1023) // 1024 * 1024
    _set_layout(cap)
    cap = _CAP
    rows_loc = cap // N_CORES
    knn_r = np.searchsorted(used, knn).astype(np.int32)        # [M, K]
    # remap into the padded row space (each core's slice has 128 pad rows)
    knn_r = (knn_r // rows_loc) * (rows_loc + 128) + (knn_r % rows_loc)

    # int8 per-row quantized context records: 128 x i8 | f16 row scale
    rowmax = np.abs(ctx_f).max(axis=1)
    rsc = np.maximum(rowmax, 1e-12) / 127.0                    # [N] f32
    q8 = np.clip(np.round(ctx_f / rsc[:, None]), -127, 127).astype(np.int8)
    tab = np.zeros((cap, ROWB), np.uint8)
    tab[:len(used), :C] = q8[used].view(np.uint8)
    tab[:len(used), C:C + 2] = rsc[used].astype(h).reshape(-1, 1) \
        .view(np.uint8)

    s = lambda g_: (np.asarray(g_, f) / np.sqrt(np.float32(1.0 + EPS_BN)))
    Wq = np.asarray(inputs["Wq"], f); Wk = np.asarray(inputs["Wk"], f)
    Wv = np.asarray(inputs["Wv"], f)
    Wp1 = np.asarray(inputs["Wp1"], f); Wp2 = np.asarray(inputs["Wp2"], f)
    Ww1 = np.asarray(inputs["Ww1"], f); Ww2 = np.asarray(inputs["Ww2"], f)

    sq = s(inputs["gq"]); bq = sq * inputs["bq"] + np.asarray(inputs["betaq"], f)
    sk = s(inputs["gk"]); bk = sk * inputs["bk"] + np.asarray(inputs["betak"], f)
    sp1 = s(inputs["gp1"])
    bp1 = sp1 * inputs["bp1"] + np.asarray(inputs["betap1"], f)
    bv = np.asarray(inputs["bv"], f) + np.asarray(inputs["bp2"], f)  # val bias
    # stacked bn for w1: row 8i+g ; fold bp2@Ww1 into bias
    sw1_g = s(inputs["gw1"])                                   # [G]
    bw1_g = (sw1_g * (np.asarray(inputs["bw1"], f)
                      + np.asarray(inputs["bp2"], f) @ Ww1)
             + np.asarray(inputs["betaw1"], f))                # [G]
    sw1 = np.tile(sw1_g, 16).astype(f)
    bw1 = np.tile(bw1_g, 16).astype(f)

    P2W1 = (Wp2 @ Ww1).astype(f)                               # [C, G]

    qf = np.asarray(inputs["query_feat"], f)
    qc = np.asarray(inputs["query_coord"], f)

    q_full = np.maximum(sq * (qf @ Wq) + bq, 0.0)          # [M, C]
    nqwT = (-(q_full @ Ww1)).T.astype(f)                   # [G, M]
    snq = np.float32(max(np.abs(nqwT).max(), 1e-12) / 127.0)
    nq8 = np.round(nqwT / snq).astype(np.int8)             # [G, M]
    fixed = {"Ww2s": Ww2, "Ww1": Ww1, "P2W1": P2W1, "Wp1": Wp1}
    fixed16 = {nm: np.asarray(v, f).astype(h).ravel() for nm, v in fixed.items()}
    # Wk|Wv|Wp2 pack, sharded through the AllGather (1/8 column-slice/core)
    pack = np.concatenate([Wk.astype(h), Wv.astype(h), Wp2.astype(h)],
                          axis=1)                              # [C, 384] f16
    packb = np.ascontiguousarray(pack).view(np.uint8)          # [C, 768]

    in_maps = []
    for c in range(N_CORES):
        sl = slice(c * M_LOC, (c + 1) * M_LOC)
        idx = knn[sl].reshape(-1)          # original ids, for coord gather
        knn_t = knn_r[sl].reshape(R_LOC // C, C).T.copy()  # renumbered, i32
        pos = (ctx_c[idx] - np.repeat(qc[sl], K, axis=0))        # [R_LOC, 3]
        psc = np.float32(max(np.abs(pos).max(), 1e-12) / 127.0)
        p8 = np.round(pos / psc).astype(np.int8).T               # [3, R_LOC]
        # fold the pos dequant scale into the positional-BN scale
        scal = np.stack([sq, bq, sk, bk, bv, sp1 * psc, bp1, sw1, bw1,
                         np.full(C, snq, f)], axis=1)            # [C, 10]
        blob = np.empty(_NTOT, h)
        pieces = dict(fixed16)
        core_tab = np.zeros((rows_loc + 128, ROWB), np.uint8)
        core_tab[:rows_loc] = tab[c * rows_loc:(c + 1) * rows_loc]
        flat = core_tab.reshape(-1)
        flat[rows_loc * ROWB:rows_loc * ROWB + 12288] = \
            packb[:, c * 96:(c + 1) * 96].reshape(-1)
        pieces["ctxtab"] = flat.view(h)
        pieces["nqwT8"] = np.ascontiguousarray(nq8[:, sl]).reshape(-1).view(h)
        pieces["knn16"] = knn_t.view(h).ravel()
        pieces["posT8"] = np.ascontiguousarray(p8).reshape(-1).view(h)
        pieces["scal"] = scal.astype(h).ravel()
        for nm, (p_, c_) in _LAYOUT:
            off = _OFFS[nm]
            blob[off:off + p_ * c_] = pieces[nm]
        in_maps.append({"blob": blob})
    return in_maps


def _get():
    global _compiled
    if _compiled is None:
        nc = _build()
        _compiled = (nc, _make_runner(nc))
    return _compiled


def _decode(res):
    """u8 per-channel quantized device output -> full [M, C] fp32."""
    outs = []
    for c in range(N_CORES):
        a = res[c]["out"]                                  # [C, M_LOC+4] u8
        m_ = np.ascontiguousarray(a[:, M_LOC:M_LOC + 4]).view(np.float32)
        y = a[:, :M_LOC].astype(np.float32)
        outs.append(((y - 128.0) * (m_ / 126.99)).T)
    return np.ascontiguousarray(np.concatenate(outs, axis=0).astype(np.float32))


def kernel(**inputs):
    in_maps = _prep_inputs(inputs)   # sets the table cap before first build
    nc, run = _get()
    return _decode(run(in_maps))
